# revision 19
# baseline (speedup 1.0000x reference)
"""Multi-head attention block (B=2, N=2048, C=1024, H=16) on 8 TRN2 NeuronCores.

Sharding (tensor-parallel over heads): core c owns global heads {2c, 2c+1}:
  - w_qkv columns for q/k/v of those heads  -> [1024, 384] slice
  - w_proj rows for those heads             -> [128, 1024] slice
  - x replicated, pre-transposed on host to xT [1024, 4096] (and cast bf16)
Each core computes a full [4096, 1024] partial of the output projection;
the host sums the 8 partials and adds b_proj.

Device pipeline per core (bf16 matmuls, fp32 PSUM accumulation):
  1. qkvT = w_slice.T @ xT -> qT/kT/vT in [head_dim, seq] layout, emitted
     as single-kt quanta through a clock-budgeted background queue.
  2. Attention per (batch, 512-wide q chunk): both heads' scores^T
     [keys=128, 512] are packed into one [128, 1024] PSUM tile via
     row-group tile_position (the K=64 matmuls run concurrently in the
     PE array), one Exp per chunk on ScalarE (1/sqrt(d) folded into the
     activation scale; no max-subtraction needed for these O(1) scores),
     then a V-matmul per head whose [keys=128, 66] stationary operand is
     [v | ones] - the ones columns make the PSUM accumulator also
     collect softmax denominators.
  3. out^T chunks feed the projection matmul directly as lhsT (k=128,
     no transpose); results stream out per [128, 512] tile.
Scheduling: the emitter runs a static clock model of the PE and ACT
engines. The exp chain is the pacer; V-matmuls are DEFERRED (pt pool
bufs=8 gives ~8 kc of elastic lag) and emitted only when their exp is
predicted complete, with background work (qkv quanta, projection
chunks) pumped into the predicted PE slack so the PE never head-of-line
blocks on the exp. Deadline markers (pump_until) remain as the
correctness net for qkv/vaug availability.
V transposes: batch 0 builds vaug via PE transposes (the DMA ring is
busy streaming xT then); batch 1 uses two whole-batch DMA xbar
transposes on the by-then-quiet ring.
Tail: per-s2 pipeline where the denominator broadcast runs as a tiny
fp32 PE matmul (ones-column outer product) instead of the slow gpsimd
partition_broadcast, so the last out-DMAs leave ~1.5us after the final
V matmul.
"""

import math
import os

import numpy as np

os.environ.setdefault("JAX_PLATFORMS", "axon,cpu")

import concourse.mybir as mybir
import concourse.tile as tile
from concourse import bacc
from concourse.bass_utils import run_bass_kernel_spmd
from concourse.masks import make_identity

F32 = mybir.dt.float32
MMDT = mybir.dt.bfloat16  # matmul operand dtype

# Problem shape (hardcoded per contract)
B, N, C, H = 2, 2048, 1024, 16
D = C // H            # 64 head dim
SEQ = B * N           # 4096
NCORES = 8
HL = H // NCORES      # 2 local heads per core
MW = 3 * HL * D       # 384 w_qkv slice cols (q|k|v for 2 heads)
KT = C // 128         # 8 contraction tiles for the projections
SC = 512              # seq chunk for qkv stage
NSC = SEQ // SC       # 8
KCN = N // 128        # 16 key chunks per batch
QW = 512              # q-chunk width for attention
NQH = N // QW         # 4
SCALE = 1.0 / math.sqrt(D)
PTB = 8               # pt pool depth = max AV lag in kc

# static clock model costs (ns)
C_EXP = 1060          # ScalarE exp of [128, 1024] from PSUM
C_SEM = 150           # cross-engine semaphore latency
C_PAIR = 330          # row-tiled scores pair (LDW + MM 512)
C_AV = 235            # one AV matmul (LDW 66 + MM 512)
C_QKT = 230           # one qkv kt-matmul (FD 512)
C_PROJ = 240          # one proj chunk matmul (FD 512)
C_VAUG0 = 520         # b=0 vaug piece (4 PE transposes)
C_CHEAP = 40          # DVE-only / DMA-only quanta


def build_nc():
    nc = bacc.Bacc("TRN2", target_bir_lowering=False, debug=False)
    xt_d = nc.dram_tensor("xt", [C, SEQ], MMDT, kind="ExternalInput")
    wqkv_d = nc.dram_tensor("wqkv", [C, MW], MMDT, kind="ExternalInput")
    wproj_d = nc.dram_tensor("wproj", [HL * D, C], MMDT, kind="ExternalInput")
    out_d = nc.dram_tensor("out", [SEQ, C], MMDT, kind="ExternalOutput")

    with tile.TileContext(nc) as tc:
        with (
            tc.tile_pool(name="const", bufs=1) as const,
            tc.tile_pool(name="qkvt", bufs=1) as qkvt,
            tc.tile_pool(name="vaugp", bufs=2) as vaugp,
            tc.tile_pool(name="ptp", bufs=PTB) as ptp,
            tc.tile_pool(name="outt", bufs=2) as outtp,
            tc.tile_pool(name="rp", bufs=2) as rp,
            tc.tile_pool(name="op", bufs=5) as op,
            tc.tile_pool(name="ps_st", bufs=2, space="PSUM") as ps_st,
            tc.tile_pool(name="ps_v", bufs=1, space="PSUM") as ps_v,
            tc.tile_pool(name="ps_aux", bufs=2, space="PSUM") as ps_aux,
        ):
            # ---- constants ----
            junk = const.tile([128, 128], MMDT, tag="junk")
            ident = const.tile([128, 128], MMDT, tag="ident")
            ones_sb = const.tile([128, 1], F32, tag="ones")
            ones_row = const.tile([1, D], F32, tag="ones_row")
            w_sb = const.tile([128, KT, MW], MMDT, tag="wqkv")
            wp_sb = const.tile([128, C], MMDT, tag="wproj")
            nc.gpsimd.memset(junk[:], 1.0)
            nc.gpsimd.memset(ones_sb[:], 1.0)
            nc.gpsimd.memset(ones_row[:], 1.0)
            make_identity(nc, ident[:])

            # persistent transposed qkv: [dim-of-2-heads=128, seq]
            q_sb = qkvt.tile([128, SEQ], MMDT, tag="q")
            k_sb = qkvt.tile([128, SEQ], MMDT, tag="k")
            v_sb = qkvt.tile([128, SEQ], MMDT, tag="v")
            dst = [q_sb, k_sb, v_sb]

            # full xT resident; chunk 0 kt-granular (so the first qkv
            # matmuls start as soon as their own kt slice lands) + weights
            # up front, the rest as whole chunks
            xt_all = qkvt.tile([128, KT, SEQ], MMDT, tag="xt_all")

            def xt_dma(sc):
                nc.sync.dma_start(
                    xt_all[:, :, sc * SC : (sc + 1) * SC],
                    xt_d.ap()[:, sc * SC : (sc + 1) * SC].rearrange(
                        "(kt p) n -> p kt n", p=128
                    ),
                )

            # input DMAs in need-order (ring processes doorbells in order):
            # k-weight columns first, then chunk 0 per-kt, q/v weights, proj
            # weights, and the rest
            wsrc = wqkv_d.ap().rearrange("(kt p) m -> p kt m", p=128)
            nc.sync.dma_start(w_sb[:, :, 128:256], wsrc[:, :, 128:256])  # k
            for kt in range(KT):
                nc.sync.dma_start(
                    xt_all[:, kt, 0:SC],
                    xt_d.ap()[:, 0:SC].rearrange("(kt p) n -> p kt n", p=128)[
                        :, kt, :
                    ],
                )
            nc.sync.dma_start(w_sb[:, :, 0:128], wsrc[:, :, 0:128])  # q
            nc.sync.dma_start(w_sb[:, :, 256:384], wsrc[:, :, 256:384])  # v
            nc.sync.dma_start(wp_sb[:], wproj_d.ap())
            for sc in range(1, NSC):
                xt_dma(sc)

            # HAM warmup: keep the PE busy during the initial DMA wait so the
            # clock gate is at 8/8 when real work lands (junk operands — no
            # dependency on any DMA). Sized to end roughly when the first
            # xt/weight slices arrive.
            wu = ps_aux.tile([128, SC], F32, tag="aux", name="wu")
            for _ in range(48):
                nc.tensor.matmul(wu[:, :128], junk[:], junk[:], start=True, stop=True)

            # ---- static clock model ----
            CLK = {"pe": 0.0, "act": 0.0}
            exp_end = {}  # global exp index -> predicted completion ns

            # ---- background queue with markers ----
            bg = []   # deadline work: (key|None, cost, closure, opens_group)
            bgl = []  # lazy work (projection chunks): (cost, closure)
            done = set()
            # True while a multi-part qkv PSUM accumulation is mid-flight;
            # lazy proj chunks share the aux tag ring and must not land
            # between two parts of a live accumulation
            grp = {"open": False}

            def run_bg_item():
                key, cost, fn, opens = bg.pop(0)
                fn()
                CLK["pe"] += cost
                grp["open"] = opens
                if key is not None:
                    done.add(key)

            def pump(n=1):
                for _ in range(n):
                    if not bg:
                        return
                    run_bg_item()

            def pump_lazy(n=1):
                for _ in range(n):
                    if not bgl:
                        return
                    cost, fn = bgl.pop(0)
                    fn()
                    CLK["pe"] += cost

            def pump_until(key):
                while key not in done:
                    run_bg_item()

            # ---- qkv + vaug emission (all via bg) ----
            def emit_qkv_part(holder, sc, m, part, nparts=8):
                if part == 0:
                    holder["ps"] = ps_aux.tile([128, SC], F32, tag="aux", name="qkv_ps")
                ps = holder["ps"]
                step = KT // nparts
                for kt in range(part * step, (part + 1) * step):
                    nc.tensor.matmul(
                        ps[:],
                        w_sb[:, kt, m * 128 : (m + 1) * 128],
                        xt_all[:, kt, sc * SC : (sc + 1) * SC],
                        start=(kt == 0),
                        stop=(kt == KT - 1),
                    )
                if part == nparts - 1:
                    if sc == 0 and m == 1:
                        # first k block: land the kc=0 key chunk first so the
                        # very first scores matmul can start early
                        nc.scalar.copy(out=dst[m][:, :128], in_=ps[:, :128])
                        nc.scalar.copy(out=dst[m][:, 128:SC], in_=ps[:, 128:])
                    elif sc == 0:
                        nc.scalar.copy(
                            out=dst[m][:, sc * SC : (sc + 1) * SC], in_=ps[:]
                        )
                    else:
                        nc.vector.tensor_copy(
                            out=dst[m][:, sc * SC : (sc + 1) * SC], in_=ps[:]
                        )

            vaug_store = {}
            vtb_store = {}
            MV = D + 2  # stationary width: v columns + ones (denominator) pad

            vtb_store[0] = vaugp.tile(
                [128, 16, D], MMDT, tag="vtb0", name="vtb", bufs=1
            )
            vtb_store[1] = vaugp.tile(
                [128, 16, D], MMDT, tag="vtb1", name="vtb", bufs=1
            )

            def emit_vtb_dma(h):
                # whole-batch transpose for b=1 on the (by then quiet) DMA
                # ring: vtb[p, T, d] = v[d, N + T*128 + p]
                nc.sync.dma_start_transpose(
                    vtb_store[h][:], v_sb[h * D : (h + 1) * D, N : 2 * N]
                )

            def emit_vaug_tr(holder, b, h, piece):
                b0 = b * N
                va = vaugp.tile([128, 4, MV], MMDT, tag=f"vaug{h}_{piece}", name="va")
                if b == 0 and piece < 3:
                    # JIT path: PE transpose (the DMA ring is busy streaming
                    # xt input during batch 0's first chunk)
                    tr = ps_aux.tile([128, 4 * D], MMDT, tag="aux", name="tr")
                    for tt in range(4):
                        t = piece * 4 + tt
                        nc.tensor.transpose(
                            tr[:, tt * D : (tt + 1) * D],
                            v_sb[h * D : (h + 1) * D, b0 + t * 128 : b0 + (t + 1) * 128],
                            ident[h * D : (h + 1) * D, h * D : (h + 1) * D],
                        )
                    nc.vector.tensor_copy(
                        out=va[:, :, :D], in_=tr[:].rearrange("p (a d) -> p a d", a=4)
                    )
                elif b == 0:
                    # piece 3 is needed late enough (kc12) that its transpose
                    # can ride the ring right after the xt stream drains;
                    # stage through vtb (b=1's later full-tile DMA overwrites)
                    nc.sync.dma_start_transpose(
                        vtb_store[h][:, 12:16, :],
                        v_sb[h * D : (h + 1) * D, b0 + 1536 : b0 + 2048],
                    )
                    nc.vector.tensor_copy(
                        out=va[:, :, :D], in_=vtb_store[h][:, 12:16, :]
                    )
                else:
                    nc.vector.tensor_copy(
                        out=va[:, :, :D],
                        in_=vtb_store[h][:, piece * 4 : (piece + 1) * 4, :],
                    )
                nc.vector.tensor_copy(
                    out=va[:, :, D : D + 2],
                    in_=ones_sb[:, None, :].to_broadcast([128, 4, 2]),
                )
                vaug_store[(b, h, piece)] = va

            def add_qkv(b, m, scl, nparts=8):
                sc = b * (NSC // 2) + scl
                holder = {}
                for part in range(nparts):
                    key = None
                    if part == nparts - 1:
                        key = ("kvq"[0 if m == 1 else 1 if m == 2 else 2], b, scl)
                    bg.append(
                        (
                            key,
                            C_QKT * (KT // nparts),
                            lambda sc=sc, m=m, part=part, holder=holder, np_=nparts: (
                                emit_qkv_part(holder, sc, m, part, np_)
                            ),
                            part < nparts - 1,
                        )
                    )

            def add_vaug(b, scl):
                for h in range(HL):
                    bg.append(
                        (
                            ("vaug", b, h, scl),
                            C_VAUG0 if (b == 0 and scl < 3) else C_CHEAP,
                            lambda b=b, h=h, scl=scl: emit_vaug_tr({}, b, h, scl),
                            False,
                        )
                    )

            # b=0: block 0 ordered for the fastest first exp (k, q, then v);
            # k-blocks early (scores need them at kc=4*blk), q for the next
            # chunk pulled ahead of late v-blocks so chunk transitions don't
            # burst
            add_qkv(0, 1, 0)
            add_qkv(0, 0, 0)
            add_qkv(0, 1, 1)
            add_qkv(0, 2, 0)
            add_vaug(0, 0)
            add_qkv(0, 1, 2)
            add_qkv(0, 2, 1)
            add_vaug(0, 1)
            add_qkv(0, 1, 3)
            add_qkv(0, 2, 2)
            add_vaug(0, 2)
            add_qkv(0, 0, 1)
            add_qkv(0, 2, 3)
            add_vaug(0, 3)
            add_qkv(0, 0, 2)
            add_qkv(0, 0, 3)
            # b=1: k/v first, whole-batch vaug transposes ride the (by now
            # quiet) DMA ring, q spread between
            add_qkv(1, 1, 0)
            add_qkv(1, 0, 0)
            add_qkv(1, 2, 0)
            add_qkv(1, 1, 1)
            add_qkv(1, 2, 1)
            add_qkv(1, 1, 2)
            add_qkv(1, 2, 2)
            add_qkv(1, 0, 1)
            add_qkv(1, 1, 3)
            add_qkv(1, 2, 3)
            # vtb DMA transposes read ALL of b=1's v — they must come after
            # every b=1 v-block is queued (the tracker cannot wait on writes
            # emitted later in program order)
            bg.append((None, C_CHEAP, lambda: emit_vtb_dma(0), False))
            bg.append((None, C_CHEAP, lambda: emit_vtb_dma(1), False))
            add_vaug(1, 0)
            add_vaug(1, 1)
            add_qkv(1, 0, 2)
            add_vaug(1, 2)
            add_vaug(1, 3)
            add_qkv(1, 0, 3)

            def emit_proj_chunk(outt, b0, s2, nck):
                pp = ps_aux.tile([128, 512], F32, tag="aux", name="proj_ps")
                nc.tensor.matmul(
                    pp[:],
                    outt[:, s2 * 128 : (s2 + 1) * 128],
                    wp_sb[:, nck * 512 : (nck + 1) * 512],
                    start=True,
                    stop=True,
                )
                o_sb = op.tile([128, 512], MMDT, tag="o", name="o_sb")
                nc.vector.tensor_copy(out=o_sb[:], in_=pp[:])
                nc.sync.dma_start(
                    out_d.ap()[
                        b0 + s2 * 128 : b0 + (s2 + 1) * 128,
                        nck * 512 : (nck + 1) * 512,
                    ],
                    o_sb[:],
                )

            # ---- attention ----
            g_scores = {}  # global kc index -> predicted scores-done ns

            def emit_scores(b, qh, kc, g):
                b0 = b * N
                q0 = b0 + qh * QW
                pump_until(("q", b, qh))
                pump_until(("k", b, kc // 4))
                st = ps_st.tile([128, 2 * QW], F32, tag="st", name="st")
                # top priority: the exp chain paces the kernel; splitting the
                # row-packed pair stalls the exp at chunk boundaries
                with tc.high_priority():
                    for h in range(HL):
                        hs = slice(h * D, (h + 1) * D)
                        nc.tensor.matmul(
                            st[:, h * QW : (h + 1) * QW],
                            k_sb[hs, b0 + kc * 128 : b0 + (kc + 1) * 128],
                            q_sb[hs, q0 : q0 + QW],
                            start=True,
                            stop=True,
                            tile_position=(h * D, 0),
                        )
                # st WAR: this buffer was read by exp g-2
                CLK["pe"] = max(CLK["pe"], exp_end.get(g - 2, 0.0)) + C_PAIR
                g_scores[g] = CLK["pe"]
                return st

            # deferred AV queue: entries emitted only when their exp is
            # predicted done (or forced by pt-pool WAR / chunk drain)
            pend_av = []
            av_done = {}  # global exp index -> PE completion of its AVs

            def emit_av(e):
                b, kc = e["b"], e["kc"]
                for h in range(HL):
                    pump_until(("vaug", b, h, kc // 4))
                for h in range(HL):
                    nc.tensor.matmul(
                        e["vps"][h][:MV, :],
                        vaug_store[(b, h, kc // 4)][:, kc % 4, :],
                        e["pt"][:, h * QW : (h + 1) * QW],
                        start=(kc == 0),
                        stop=(kc == KCN - 1),
                    )
                CLK["pe"] = max(CLK["pe"], exp_end[e["g"]]) + 2 * C_AV
                av_done[e["g"]] = CLK["pe"]
                if e["fin"] is not None:
                    e["fin"]()

            def flush_av(n=None):
                cnt = len(pend_av) if n is None else n
                for _ in range(cnt):
                    if not pend_av:
                        return
                    emit_av(pend_av.pop(0))

            chunks = [(b, qh) for b in range(B) for qh in range(NQH)]
            pend = emit_scores(0, 0, 0, 0)
            outt = outu = None
            rs2 = None
            vps_cur = None
            for ci, (b, qh) in enumerate(chunks):
                b0 = b * N
                last = ci == len(chunks) - 1
                if qh == 0:
                    outt = outtp.tile([128, N], MMDT, tag="outT", name="outt")
                    outu = [
                        outtp.tile([D, N], MMDT, tag=f"outu{h}", name="outu")
                        for h in range(HL)
                    ]
                    # [1, qh, h, q] so each chunk's denominator block is
                    # contiguous (2D-viewable for partition_broadcast)
                    rs2 = rp.tile([1, NQH, HL, QW], F32, tag="rs2", name="rs2")
                vps_cur = [
                    ps_v.tile([128, QW], F32, tag=f"vps{h}", name=f"vps{h}")
                    for h in range(HL)
                ]
                qs = slice(qh * QW, (qh + 1) * QW)

                # per-chunk finalizer, attached to the last AV of the chunk:
                # drain vps to SBUF, normalize off the critical path, queue
                # this q-range's projection chunks as lazy work
                def make_fin(
                    b=b, qh=qh, b0=b0, outt=outt, outu=outu, rs2=rs2,
                    vps=vps_cur, qs=qs, last=last,
                ):
                    def fin():
                        if last:
                            return  # tail handled inline below
                        for h in range(HL):
                            nc.vector.tensor_copy(out=outu[h][:, qs], in_=vps[h][:D, :])
                            nc.vector.tensor_copy(
                                out=rs2[:, qh, h, :], in_=vps[h][D : D + 1, :]
                            )
                        # one broadcast+recip covers both heads' denominators
                        rb = rp.tile([D, HL * QW], F32, tag="rb", name="rb")
                        nc.gpsimd.partition_broadcast(
                            rb[:], rs2[:, qh].rearrange("p h q -> p (h q)")
                        )
                        rbr = rp.tile([D, HL * QW], F32, tag="rbr", name="rbr")
                        nc.vector.reciprocal_approx_fast(out=rbr[:], in_=rb[:])
                        for h in range(HL):
                            nc.vector.tensor_mul(
                                out=outt[h * D : (h + 1) * D, qs],
                                in0=outu[h][:, qs],
                                in1=rbr[:, h * QW : (h + 1) * QW],
                            )
                        for s2 in range(qh * (QW // 128), (qh + 1) * (QW // 128)):
                            for nck in range(C // 512):
                                bgl.append(
                                    (
                                        C_PROJ,
                                        lambda outt=outt, b0=b0, s2=s2, nck=nck: (
                                            emit_proj_chunk(outt, b0, s2, nck)
                                        ),
                                    )
                                )
                    return fin

                fin_cb = make_fin()
                for kc in range(KCN):
                    g = ci * KCN + kc
                    if kc + 1 < KCN:
                        nxt = emit_scores(b, qh, kc + 1, g + 1)
                    elif ci + 1 < len(chunks):
                        nb, nqh = chunks[ci + 1]
                        nxt = emit_scores(nb, nqh, 0, g + 1)
                    else:
                        nxt = None

                    # pt-pool WAR: the buffer exp(g) writes was read by the
                    # AVs of exp g-PTB+1; those must be emitted first
                    while pend_av and pend_av[0]["g"] <= g - (PTB - 1):
                        emit_av(pend_av.pop(0))

                    pt = ptp.tile([128, 2 * QW], MMDT, tag="pt")
                    nc.scalar.activation(
                        out=pt[:],
                        in_=pend[:],
                        func=mybir.ActivationFunctionType.Exp,
                        scale=SCALE,
                    )
                    e_end = (
                        max(
                            CLK["act"],
                            g_scores[g] + C_SEM,
                            av_done.get(g - PTB, 0.0) + C_SEM,
                        )
                        + C_EXP
                    )
                    CLK["act"] = e_end
                    exp_end[g] = e_end
                    pend_av.append(
                        {
                            "b": b, "kc": kc, "g": g, "pt": pt,
                            "vps": vps_cur, "fin": fin_cb if kc == KCN - 1 else None,
                        }
                    )

                    # first drain every ripe AV (its exp is comfortably done —
                    # the PE never stalls on these and they gate the exp
                    # chain via the pt-pool WAR horizon)
                    while pend_av and exp_end[pend_av[0]["g"]] <= CLK["pe"] - 150.0:
                        emit_av(pend_av.pop(0))

                    # then pace background work against the ACT frontier,
                    # with a backlog-pressure term so the queues drain evenly
                    # across the remaining kcs instead of piling into a tail.
                    # Lazy proj chunks run only once the qkv queue is empty —
                    # they share the aux PSUM ring with the qkv accumulators
                    # and interleaving costs WAR stalls.
                    rem_kc = max(len(chunks) * KCN - 1 - g, 1)
                    rem_cost = sum(it[1] for it in bg) + sum(c for c, _ in bgl)
                    over = max(0.0, (rem_cost - rem_kc * 260.0) / rem_kc)
                    slack_until = exp_end[g] - 150.0 + over
                    if ci == len(chunks) - 1:
                        slack_until += 800.0
                    while CLK["pe"] < slack_until:
                        if pend_av and exp_end[pend_av[0]["g"]] <= CLK["pe"] - 150.0:
                            emit_av(pend_av.pop(0))
                        elif bg:
                            run_bg_item()
                        elif bgl:
                            pump_lazy(1)
                        else:
                            break

                    pend = nxt

                if last:
                    # flush every remaining AV (and any leftover background)
                    flush_av()
                    while bg:
                        pump(1)
                    pump_lazy(len(bgl))
                    # tail: fine-grained per-s2 pipeline — PE ones-column
                    # matmul broadcasts the two denominators (fp32, ~0.4us
                    # cheaper than gpsimd partition_broadcast), recip, two
                    # muls, then this s2's projections and out-DMAs
                    nc.scalar.copy(out=rs2[:, qh, 0, :], in_=vps_cur[0][D : D + 1, :])
                    nc.vector.tensor_copy(
                        out=rs2[:, qh, 1, :], in_=vps_cur[1][D : D + 1, :]
                    )
                    for s2l in range(QW // 128):
                        s2 = qh * (QW // 128) + s2l
                        cs = slice(s2l * 128, (s2l + 1) * 128)
                        qsl = slice(
                            qh * QW + s2l * 128, qh * QW + (s2l + 1) * 128
                        )
                        rbp = ps_aux.tile([128, SC], F32, tag="aux", name="rbp")
                        nc.tensor.matmul(
                            rbp[:D, : HL * 128],
                            ones_row[:, :],
                            rs2[:, qh, :, s2l * 128 : (s2l + 1) * 128],
                            start=True,
                            stop=True,
                        )
                        rbr = rp.tile([D, HL * 128], F32, tag="rbrs", name="rbrs")
                        nc.vector.reciprocal_approx_fast(
                            out=rbr[:], in_=rbp[:D, : HL * 128]
                        )
                        for h in range(HL):
                            nc.vector.tensor_mul(
                                out=outt[h * D : (h + 1) * D, qsl],
                                in0=vps_cur[h][:D, cs],
                                in1=rbr[:, h * 128 : (h + 1) * 128],
                            )
                        # both nck projections into one (now free) score tile,
                        # then the two drain copies run on Scalar and Vector
                        # in parallel
                        pp2 = ps_st.tile([128, 2 * QW], F32, tag="st", name="tp")
                        for nck in range(C // 512):
                            nc.tensor.matmul(
                                pp2[:, nck * 512 : (nck + 1) * 512],
                                outt[:, s2 * 128 : (s2 + 1) * 128],
                                wp_sb[:, nck * 512 : (nck + 1) * 512],
                                start=True,
                                stop=True,
                            )
                        for nck in range(C // 512):
                            o_sb = op.tile([128, 512], MMDT, tag="o", name="o_sb")
                            src = pp2[:, nck * 512 : (nck + 1) * 512]
                            if nck == 0:
                                nc.scalar.copy(out=o_sb[:], in_=src)
                            else:
                                nc.vector.tensor_copy(out=o_sb[:], in_=src)
                            nc.sync.dma_start(
                                out_d.ap()[
                                    b0 + s2 * 128 : b0 + (s2 + 1) * 128,
                                    nck * 512 : (nck + 1) * 512,
                                ],
                                o_sb[:],
                            )

            # drain remaining background work
            flush_av()
            while bg:
                pump(1)
            pump_lazy(len(bgl))
    nc.compile()
    return nc


_NC_CACHE = {}


def _get_nc():
    if "nc" not in _NC_CACHE:
        _NC_CACHE["nc"] = build_nc()
    return _NC_CACHE["nc"]


def make_in_maps(x, w_qkv, w_proj):
    np_dt = mybir.dt.np(MMDT)
    x = np.asarray(x, dtype=np.float32)
    w_qkv = np.asarray(w_qkv, dtype=np.float32)
    w_proj = np.asarray(w_proj, dtype=np.float32)
    xt = np.ascontiguousarray(x.reshape(SEQ, C).T.astype(np_dt))
    in_maps = []
    for c in range(NCORES):
        cs = slice(128 * c, 128 * c + 128)
        wslice = np.ascontiguousarray(
            np.concatenate(
                [w_qkv[:, cs], w_qkv[:, C:][:, cs], w_qkv[:, 2 * C :][:, cs]], axis=1
            ).astype(np_dt)
        )
        in_maps.append(
            {
                "xt": xt,
                "wqkv": wslice,
                "wproj": np.ascontiguousarray(w_proj[cs, :].astype(np_dt)),
            }
        )
    return in_maps


def kernel(x, w_qkv, w_proj, b_proj, _run_kwargs=None):
    # snapshot inputs to host numpy before any device/compile interaction
    in_maps = make_in_maps(x, w_qkv, w_proj)
    b_proj = np.asarray(b_proj, dtype=np.float32)
    nc = _get_nc()
    res = run_bass_kernel_spmd(
        nc, in_maps, core_ids=list(range(NCORES)), **(_run_kwargs or {})
    )
    acc = res.results[0]["out"].astype(np.float32)
    for c in range(1, NCORES):
        acc = acc + res.results[c]["out"]
    acc = acc + np.asarray(b_proj, dtype=np.float32)[None, :]
    out = acc.reshape(B, N, C)
    if _run_kwargs:
        kernel.last_result = res
    return out


# revision 21
# speedup vs baseline: 1.0342x; 1.0342x over previous
"""Multi-head attention block (B=2, N=2048, C=1024, H=16) on 8 TRN2 NeuronCores.

Sharding (tensor-parallel over heads): core c owns global heads {2c, 2c+1}:
  - w_qkv columns for q/k/v of those heads  -> [1024, 384] slice
  - w_proj rows for those heads             -> [128, 1024] slice
  - x replicated, pre-transposed on host to xT [1024, 4096] (and cast bf16)
Each core computes a full [4096, 1024] partial of the output projection;
the host sums the 8 partials and adds b_proj.

Device pipeline per core (bf16 matmuls, fp32 PSUM accumulation):
  1. qkvT = w_slice.T @ xT -> qT/kT/vT in [head_dim, seq] layout, emitted
     as single-kt quanta through a clock-budgeted background queue.
  2. Attention per (batch, 512-wide q chunk): both heads' scores^T
     [keys=128, 512] are packed into one [128, 1024] PSUM tile via
     row-group tile_position (the K=64 matmuls run concurrently in the
     PE array), one Exp per chunk on ScalarE (1/sqrt(d) folded into the
     activation scale; no max-subtraction needed for these O(1) scores),
     then a V-matmul per head whose [keys=128, 66] stationary operand is
     [v | ones] - the ones columns make the PSUM accumulator also
     collect softmax denominators.
  3. out^T chunks feed the projection matmul directly as lhsT (k=128,
     no transpose); results stream out per [128, 512] tile.
Scheduling: the emitter runs a static clock model of the PE and ACT
engines. The exp chain is the pacer; V-matmuls are DEFERRED (pt pool
bufs=8 gives ~8 kc of elastic lag) and emitted only when their exp is
predicted complete, with background work (qkv quanta, projection
chunks) pumped into the predicted PE slack so the PE never head-of-line
blocks on the exp. Deadline markers (pump_until) remain as the
correctness net for qkv/vaug availability.
V transposes: batch 0 builds vaug via PE transposes (the DMA ring is
busy streaming xT then); batch 1 uses two whole-batch DMA xbar
transposes on the by-then-quiet ring.
Tail: per-s2 pipeline where the denominator broadcast runs as a tiny
fp32 PE matmul (ones-column outer product) instead of the slow gpsimd
partition_broadcast, so the last out-DMAs leave ~1.5us after the final
V matmul.
"""

import math
import os

import numpy as np

os.environ.setdefault("JAX_PLATFORMS", "axon,cpu")

import concourse.mybir as mybir
import concourse.tile as tile
from concourse import bacc
from concourse.bass_utils import run_bass_kernel_spmd
from concourse.masks import make_identity

F32 = mybir.dt.float32
MMDT = mybir.dt.bfloat16  # matmul operand dtype

# Problem shape (hardcoded per contract)
B, N, C, H = 2, 2048, 1024, 16
D = C // H            # 64 head dim
SEQ = B * N           # 4096
NCORES = 8
HL = H // NCORES      # 2 local heads per core
MW = 3 * HL * D       # 384 w_qkv slice cols (q|k|v for 2 heads)
KT = C // 128         # 8 contraction tiles for the projections
SC = 512              # seq chunk for qkv stage
NSC = SEQ // SC       # 8
KCN = N // 128        # 16 key chunks per batch
QW = 512              # q-chunk width for attention
NQH = N // QW         # 4
SCALE = 1.0 / math.sqrt(D)
PTB = 8               # pt pool depth = max AV lag in kc

# static clock model costs (ns)
C_EXP = 1060          # ScalarE exp of [128, 1024] from PSUM
C_SEM = 150           # cross-engine semaphore latency
C_PAIR = 330          # row-tiled scores pair (LDW + MM 512)
C_AV = 235            # one AV matmul (LDW 66 + MM 512)
C_QKT = 230           # one qkv kt-matmul (FD 512)
C_PROJ = 240          # one proj chunk matmul (FD 512)
C_VAUG0 = 520         # b=0 vaug piece (4 PE transposes)
C_CHEAP = 40          # DVE-only / DMA-only quanta


def build_nc():
    nc = bacc.Bacc("TRN2", target_bir_lowering=False, debug=False)
    xt_d = nc.dram_tensor("xt", [C, SEQ], MMDT, kind="ExternalInput")
    wqkv_d = nc.dram_tensor("wqkv", [C, MW], MMDT, kind="ExternalInput")
    wproj_d = nc.dram_tensor("wproj", [HL * D, C], MMDT, kind="ExternalInput")
    out_d = nc.dram_tensor("out", [SEQ, C], MMDT, kind="ExternalOutput")

    with tile.TileContext(nc) as tc:
        with (
            tc.tile_pool(name="const", bufs=1) as const,
            tc.tile_pool(name="qkvt", bufs=1) as qkvt,
            tc.tile_pool(name="vaugp", bufs=2) as vaugp,
            tc.tile_pool(name="ptp", bufs=PTB) as ptp,
            tc.tile_pool(name="outt", bufs=2) as outtp,
            tc.tile_pool(name="rp", bufs=2) as rp,
            tc.tile_pool(name="op", bufs=5) as op,
            tc.tile_pool(name="ps_st", bufs=2, space="PSUM") as ps_st,
            tc.tile_pool(name="ps_v", bufs=1, space="PSUM") as ps_v,
            tc.tile_pool(name="ps_aux", bufs=2, space="PSUM") as ps_aux,
        ):
            # ---- constants ----
            junk = const.tile([128, 128], MMDT, tag="junk")
            ident = const.tile([128, 128], MMDT, tag="ident")
            ones_sb = const.tile([128, 1], F32, tag="ones")
            ones_row = const.tile([1, D], F32, tag="ones_row")
            w_sb = const.tile([128, KT, MW], MMDT, tag="wqkv")
            wp_sb = const.tile([128, C], MMDT, tag="wproj")
            nc.gpsimd.memset(junk[:], 1.0)
            nc.gpsimd.memset(ones_sb[:], 1.0)
            nc.gpsimd.memset(ones_row[:], 1.0)
            make_identity(nc, ident[:])

            # persistent transposed qkv: [dim-of-2-heads=128, seq]
            q_sb = qkvt.tile([128, SEQ], MMDT, tag="q")
            k_sb = qkvt.tile([128, SEQ], MMDT, tag="k")
            v_sb = qkvt.tile([128, SEQ], MMDT, tag="v")
            dst = [q_sb, k_sb, v_sb]

            # full xT resident; chunk 0 kt-granular (so the first qkv
            # matmuls start as soon as their own kt slice lands) + weights
            # up front, the rest as whole chunks
            xt_all = qkvt.tile([128, KT, SEQ], MMDT, tag="xt_all")

            def xt_dma(sc):
                nc.sync.dma_start(
                    xt_all[:, :, sc * SC : (sc + 1) * SC],
                    xt_d.ap()[:, sc * SC : (sc + 1) * SC].rearrange(
                        "(kt p) n -> p kt n", p=128
                    ),
                )

            # input DMAs in need-order (ring processes doorbells in order):
            # k-weight columns first, then chunk 0 per-kt, q/v weights, proj
            # weights, and the rest
            def xt_dma_kts(sc):
                src = xt_d.ap()[:, sc * SC : (sc + 1) * SC].rearrange(
                    "(kt p) n -> p kt n", p=128
                )
                for kt in range(KT):
                    nc.sync.dma_start(
                        xt_all[:, kt, sc * SC : (sc + 1) * SC], src[:, kt, :]
                    )

            wsrc = wqkv_d.ap().rearrange("(kt p) m -> p kt m", p=128)
            nc.sync.dma_start(w_sb[:, :, 128:256], wsrc[:, :, 128:256])  # k
            xt_dma_kts(0)
            nc.sync.dma_start(w_sb[:, :, 0:128], wsrc[:, :, 0:128])  # q
            xt_dma_kts(1)
            nc.sync.dma_start(w_sb[:, :, 256:384], wsrc[:, :, 256:384])  # v
            xt_dma_kts(2)
            nc.sync.dma_start(wp_sb[:], wproj_d.ap())
            xt_dma_kts(3)
            for sc in range(4, NSC):
                xt_dma(sc)

            # HAM warmup: keep the PE busy during the initial DMA wait so the
            # clock gate is at 8/8 when real work lands (junk operands — no
            # dependency on any DMA). Sized to end roughly when the first
            # xt/weight slices arrive.
            wu = ps_aux.tile([128, SC], F32, tag="aux", name="wu")
            for _ in range(32):
                nc.tensor.matmul(wu[:, :128], junk[:], junk[:], start=True, stop=True)

            # ---- static clock model ----
            CLK = {"pe": 0.0, "act": 0.0}
            exp_end = {}  # global exp index -> predicted completion ns

            # ---- background queue with markers ----
            bg = []   # deadline work: (key|None, cost, closure, opens_group)
            bgl = []  # lazy work (projection chunks): (cost, closure)
            done = set()
            # True while a multi-part qkv PSUM accumulation is mid-flight;
            # lazy proj chunks share the aux tag ring and must not land
            # between two parts of a live accumulation
            grp = {"open": False}

            def run_bg_item():
                key, cost, fn, opens = bg.pop(0)
                fn()
                CLK["pe"] += cost
                grp["open"] = opens
                if key is not None:
                    done.add(key)

            def pump(n=1):
                for _ in range(n):
                    if not bg:
                        return
                    run_bg_item()

            def pump_lazy(n=1):
                for _ in range(n):
                    if not bgl:
                        return
                    cost, fn = bgl.pop(0)
                    fn()
                    CLK["pe"] += cost

            def pump_until(key):
                while key not in done:
                    run_bg_item()

            # ---- qkv + vaug emission (all via bg) ----
            def emit_qkv_part(holder, sc, m, part, nparts=2):
                if part == 0:
                    holder["ps"] = ps_aux.tile([128, SC], F32, tag="aux", name="qkv_ps")
                ps = holder["ps"]
                step = KT // nparts
                for kt in range(part * step, (part + 1) * step):
                    nc.tensor.matmul(
                        ps[:],
                        w_sb[:, kt, m * 128 : (m + 1) * 128],
                        xt_all[:, kt, sc * SC : (sc + 1) * SC],
                        start=(kt == 0),
                        stop=(kt == KT - 1),
                    )
                if part == nparts - 1:
                    if sc == 0 and m == 1:
                        # first k block: land the kc=0 key chunk first so the
                        # very first scores matmul can start early
                        nc.scalar.copy(out=dst[m][:, :128], in_=ps[:, :128])
                        nc.scalar.copy(out=dst[m][:, 128:SC], in_=ps[:, 128:])
                    elif sc == 0:
                        nc.scalar.copy(
                            out=dst[m][:, sc * SC : (sc + 1) * SC], in_=ps[:]
                        )
                    else:
                        nc.vector.tensor_copy(
                            out=dst[m][:, sc * SC : (sc + 1) * SC], in_=ps[:]
                        )

            vaug_store = {}
            vtb_store = {}
            MV = D + 2  # stationary width: v columns + ones (denominator) pad

            vtb_store[0] = vaugp.tile(
                [128, 16, D], MMDT, tag="vtb0", name="vtb", bufs=1
            )
            vtb_store[1] = vaugp.tile(
                [128, 16, D], MMDT, tag="vtb1", name="vtb", bufs=1
            )

            def emit_vtb_dma(h):
                # whole-batch transpose for b=1 on the (by then quiet) DMA
                # ring: vtb[p, T, d] = v[d, N + T*128 + p]
                nc.sync.dma_start_transpose(
                    vtb_store[h][:], v_sb[h * D : (h + 1) * D, N : 2 * N]
                )

            def emit_vaug_tr(holder, b, h, piece):
                b0 = b * N
                va = vaugp.tile([128, 4, MV], MMDT, tag=f"vaug{h}_{piece}", name="va")
                if b == 0 and piece < 3:
                    # JIT path: PE transpose (the DMA ring is busy streaming
                    # xt input during batch 0's first chunk)
                    tr = ps_aux.tile([128, 4 * D], MMDT, tag="aux", name="tr")
                    for tt in range(4):
                        t = piece * 4 + tt
                        nc.tensor.transpose(
                            tr[:, tt * D : (tt + 1) * D],
                            v_sb[h * D : (h + 1) * D, b0 + t * 128 : b0 + (t + 1) * 128],
                            ident[h * D : (h + 1) * D, h * D : (h + 1) * D],
                        )
                    nc.vector.tensor_copy(
                        out=va[:, :, :D], in_=tr[:].rearrange("p (a d) -> p a d", a=4)
                    )
                elif b == 0:
                    # piece 3 is needed late enough (kc12) that its transpose
                    # can ride the ring right after the xt stream drains;
                    # stage through vtb (b=1's later full-tile DMA overwrites)
                    nc.sync.dma_start_transpose(
                        vtb_store[h][:, 12:16, :],
                        v_sb[h * D : (h + 1) * D, b0 + 1536 : b0 + 2048],
                    )
                    nc.vector.tensor_copy(
                        out=va[:, :, :D], in_=vtb_store[h][:, 12:16, :]
                    )
                else:
                    nc.vector.tensor_copy(
                        out=va[:, :, :D],
                        in_=vtb_store[h][:, piece * 4 : (piece + 1) * 4, :],
                    )
                nc.vector.tensor_copy(
                    out=va[:, :, D : D + 2],
                    in_=ones_sb[:, None, :].to_broadcast([128, 4, 2]),
                )
                vaug_store[(b, h, piece)] = va

            def add_qkv(b, m, scl, nparts=2):
                sc = b * (NSC // 2) + scl
                holder = {}
                for part in range(nparts):
                    key = None
                    if part == nparts - 1:
                        key = ("kvq"[0 if m == 1 else 1 if m == 2 else 2], b, scl)
                    bg.append(
                        (
                            key,
                            C_QKT * (KT // nparts),
                            lambda sc=sc, m=m, part=part, holder=holder, np_=nparts: (
                                emit_qkv_part(holder, sc, m, part, np_)
                            ),
                            part < nparts - 1,
                        )
                    )

            def add_vaug(b, scl):
                for h in range(HL):
                    bg.append(
                        (
                            ("vaug", b, h, scl),
                            C_VAUG0 if (b == 0 and scl < 3) else C_CHEAP,
                            lambda b=b, h=h, scl=scl: emit_vaug_tr({}, b, h, scl),
                            False,
                        )
                    )

            # b=0: block 0 ordered for the fastest first exp (k, q, then v);
            # k-blocks early (scores need them at kc=4*blk), q for the next
            # chunk pulled ahead of late v-blocks so chunk transitions don't
            # burst
            add_qkv(0, 1, 0)
            add_qkv(0, 0, 0)
            add_qkv(0, 1, 1)
            add_qkv(0, 2, 0)
            add_vaug(0, 0)
            add_qkv(0, 1, 2)
            add_qkv(0, 2, 1)
            add_vaug(0, 1)
            add_qkv(0, 1, 3)
            add_qkv(0, 2, 2)
            add_vaug(0, 2)
            add_qkv(0, 0, 1)
            add_qkv(0, 2, 3)
            add_vaug(0, 3)
            add_qkv(0, 0, 2)
            add_qkv(0, 0, 3)
            # b=1: k/v first, whole-batch vaug transposes ride the (by now
            # quiet) DMA ring, q spread between
            add_qkv(1, 1, 0)
            add_qkv(1, 0, 0)
            add_qkv(1, 2, 0)
            add_qkv(1, 1, 1)
            add_qkv(1, 2, 1)
            add_qkv(1, 1, 2)
            add_qkv(1, 2, 2)
            add_qkv(1, 0, 1)
            add_qkv(1, 1, 3)
            add_qkv(1, 2, 3)
            # vtb DMA transposes read ALL of b=1's v — they must come after
            # every b=1 v-block is queued (the tracker cannot wait on writes
            # emitted later in program order)
            bg.append((None, C_CHEAP, lambda: emit_vtb_dma(0), False))
            bg.append((None, C_CHEAP, lambda: emit_vtb_dma(1), False))
            add_vaug(1, 0)
            add_vaug(1, 1)
            add_qkv(1, 0, 2)
            add_vaug(1, 2)
            add_vaug(1, 3)
            add_qkv(1, 0, 3)

            def emit_proj_chunk(outt, b0, s2, nck):
                pp = ps_aux.tile([128, 512], F32, tag="aux", name="proj_ps")
                nc.tensor.matmul(
                    pp[:],
                    outt[:, s2 * 128 : (s2 + 1) * 128],
                    wp_sb[:, nck * 512 : (nck + 1) * 512],
                    start=True,
                    stop=True,
                )
                o_sb = op.tile([128, 512], MMDT, tag="o", name="o_sb")
                nc.vector.tensor_copy(out=o_sb[:], in_=pp[:])
                nc.sync.dma_start(
                    out_d.ap()[
                        b0 + s2 * 128 : b0 + (s2 + 1) * 128,
                        nck * 512 : (nck + 1) * 512,
                    ],
                    o_sb[:],
                )

            # ---- attention ----
            g_scores = {}  # global kc index -> predicted scores-done ns

            def emit_scores(b, qh, kc, g):
                b0 = b * N
                q0 = b0 + qh * QW
                pump_until(("q", b, qh))
                pump_until(("k", b, kc // 4))
                st = ps_st.tile([128, 2 * QW], F32, tag="st", name="st")
                # top priority: the exp chain paces the kernel; splitting the
                # row-packed pair stalls the exp at chunk boundaries
                with tc.high_priority():
                    for h in range(HL):
                        hs = slice(h * D, (h + 1) * D)
                        nc.tensor.matmul(
                            st[:, h * QW : (h + 1) * QW],
                            k_sb[hs, b0 + kc * 128 : b0 + (kc + 1) * 128],
                            q_sb[hs, q0 : q0 + QW],
                            start=True,
                            stop=True,
                            tile_position=(h * D, 0),
                        )
                # st WAR: this buffer was read by exp g-2
                CLK["pe"] = max(CLK["pe"], exp_end.get(g - 2, 0.0)) + C_PAIR
                g_scores[g] = CLK["pe"]
                return st

            # deferred AV queue: entries emitted only when their exp is
            # predicted done (or forced by pt-pool WAR / chunk drain)
            pend_av = []
            av_done = {}  # global exp index -> PE completion of its AVs

            def emit_av(e):
                b, kc = e["b"], e["kc"]
                for h in range(HL):
                    pump_until(("vaug", b, h, kc // 4))
                for h in range(HL):
                    nc.tensor.matmul(
                        e["vps"][h][:MV, :],
                        vaug_store[(b, h, kc // 4)][:, kc % 4, :],
                        e["pt"][:, h * QW : (h + 1) * QW],
                        start=(kc == 0),
                        stop=(kc == KCN - 1),
                    )
                CLK["pe"] = max(CLK["pe"], exp_end[e["g"]]) + 2 * C_AV
                av_done[e["g"]] = CLK["pe"]
                if e["fin"] is not None:
                    e["fin"]()

            def flush_av(n=None):
                cnt = len(pend_av) if n is None else n
                for _ in range(cnt):
                    if not pend_av:
                        return
                    emit_av(pend_av.pop(0))

            chunks = [(b, qh) for b in range(B) for qh in range(NQH)]
            pend = emit_scores(0, 0, 0, 0)
            outt = outu = None
            rs2 = None
            vps_cur = None
            for ci, (b, qh) in enumerate(chunks):
                b0 = b * N
                last = ci == len(chunks) - 1
                if qh == 0:
                    outt = outtp.tile([128, N], MMDT, tag="outT", name="outt")
                    outu = [
                        outtp.tile([D, N], MMDT, tag=f"outu{h}", name="outu")
                        for h in range(HL)
                    ]
                    # [1, qh, h, q] so each chunk's denominator block is
                    # contiguous (2D-viewable for partition_broadcast)
                    rs2 = rp.tile([1, NQH, HL, QW], F32, tag="rs2", name="rs2")
                vps_cur = [
                    ps_v.tile([128, QW], F32, tag=f"vps{h}", name=f"vps{h}")
                    for h in range(HL)
                ]
                qs = slice(qh * QW, (qh + 1) * QW)

                # per-chunk finalizer, attached to the last AV of the chunk:
                # drain vps to SBUF, normalize off the critical path, queue
                # this q-range's projection chunks as lazy work
                def make_fin(
                    b=b, qh=qh, b0=b0, outt=outt, outu=outu, rs2=rs2,
                    vps=vps_cur, qs=qs, last=last,
                ):
                    def fin():
                        if last:
                            return  # tail handled inline below
                        for h in range(HL):
                            nc.vector.tensor_copy(out=outu[h][:, qs], in_=vps[h][:D, :])
                            nc.vector.tensor_copy(
                                out=rs2[:, qh, h, :], in_=vps[h][D : D + 1, :]
                            )
                        # one broadcast+recip covers both heads' denominators
                        rb = rp.tile([D, HL * QW], F32, tag="rb", name="rb")
                        nc.gpsimd.partition_broadcast(
                            rb[:], rs2[:, qh].rearrange("p h q -> p (h q)")
                        )
                        rbr = rp.tile([D, HL * QW], F32, tag="rbr", name="rbr")
                        nc.vector.reciprocal_approx_fast(out=rbr[:], in_=rb[:])
                        for h in range(HL):
                            nc.vector.tensor_mul(
                                out=outt[h * D : (h + 1) * D, qs],
                                in0=outu[h][:, qs],
                                in1=rbr[:, h * QW : (h + 1) * QW],
                            )
                        for s2 in range(qh * (QW // 128), (qh + 1) * (QW // 128)):
                            for nck in range(C // 512):
                                bgl.append(
                                    (
                                        C_PROJ,
                                        lambda outt=outt, b0=b0, s2=s2, nck=nck: (
                                            emit_proj_chunk(outt, b0, s2, nck)
                                        ),
                                    )
                                )
                    return fin

                fin_cb = make_fin()
                for kc in range(KCN):
                    g = ci * KCN + kc
                    if kc + 1 < KCN:
                        nxt = emit_scores(b, qh, kc + 1, g + 1)
                    elif ci + 1 < len(chunks):
                        nb, nqh = chunks[ci + 1]
                        nxt = emit_scores(nb, nqh, 0, g + 1)
                    else:
                        nxt = None

                    # pt-pool WAR: the buffer exp(g) writes was read by the
                    # AVs of exp g-PTB+1; those must be emitted first
                    while pend_av and pend_av[0]["g"] <= g - (PTB - 1):
                        emit_av(pend_av.pop(0))

                    pt = ptp.tile([128, 2 * QW], MMDT, tag="pt")
                    nc.scalar.activation(
                        out=pt[:],
                        in_=pend[:],
                        func=mybir.ActivationFunctionType.Exp,
                        scale=SCALE,
                    )
                    e_end = (
                        max(
                            CLK["act"],
                            g_scores[g] + C_SEM,
                            av_done.get(g - PTB, 0.0) + C_SEM,
                        )
                        + C_EXP
                    )
                    CLK["act"] = e_end
                    exp_end[g] = e_end
                    pend_av.append(
                        {
                            "b": b, "kc": kc, "g": g, "pt": pt,
                            "vps": vps_cur, "fin": fin_cb if kc == KCN - 1 else None,
                        }
                    )

                    # first drain every ripe AV (its exp is comfortably done —
                    # the PE never stalls on these and they gate the exp
                    # chain via the pt-pool WAR horizon)
                    while pend_av and exp_end[pend_av[0]["g"]] <= CLK["pe"] - 150.0:
                        emit_av(pend_av.pop(0))

                    # then pace background work against the ACT frontier,
                    # with a backlog-pressure term so the queues drain evenly
                    # across the remaining kcs instead of piling into a tail.
                    # Lazy proj chunks run only once the qkv queue is empty —
                    # they share the aux PSUM ring with the qkv accumulators
                    # and interleaving costs WAR stalls.
                    rem_kc = max(len(chunks) * KCN - 1 - g, 1)
                    rem_cost = sum(it[1] for it in bg) + sum(c for c, _ in bgl)
                    over = max(0.0, (rem_cost - rem_kc * 230.0) / min(rem_kc, 48))
                    slack_until = exp_end[g] - 150.0 + over
                    if ci == len(chunks) - 1:
                        slack_until += 800.0
                    while CLK["pe"] < slack_until:
                        if pend_av and exp_end[pend_av[0]["g"]] <= CLK["pe"] - 150.0:
                            emit_av(pend_av.pop(0))
                        elif bg:
                            run_bg_item()
                        elif bgl:
                            pump_lazy(1)
                        else:
                            break

                    pend = nxt

                if last:
                    # flush every remaining AV (and any leftover background)
                    flush_av()
                    while bg:
                        pump(1)
                    pump_lazy(len(bgl))
                    # tail: fine-grained per-s2 pipeline — PE ones-column
                    # matmul broadcasts the two denominators (fp32, ~0.4us
                    # cheaper than gpsimd partition_broadcast), recip, two
                    # muls, then this s2's projections and out-DMAs
                    nc.scalar.copy(out=rs2[:, qh, 0, :], in_=vps_cur[0][D : D + 1, :])
                    nc.vector.tensor_copy(
                        out=rs2[:, qh, 1, :], in_=vps_cur[1][D : D + 1, :]
                    )
                    for s2l in range(QW // 128):
                        s2 = qh * (QW // 128) + s2l
                        cs = slice(s2l * 128, (s2l + 1) * 128)
                        qsl = slice(
                            qh * QW + s2l * 128, qh * QW + (s2l + 1) * 128
                        )
                        rbp = ps_aux.tile([128, SC], F32, tag="aux", name="rbp")
                        nc.tensor.matmul(
                            rbp[:D, : HL * 128],
                            ones_row[:, :],
                            rs2[:, qh, :, s2l * 128 : (s2l + 1) * 128],
                            start=True,
                            stop=True,
                        )
                        rbr = rp.tile([D, HL * 128], F32, tag="rbrs", name="rbrs")
                        nc.vector.reciprocal_approx_fast(
                            out=rbr[:], in_=rbp[:D, : HL * 128]
                        )
                        for h in range(HL):
                            nc.vector.tensor_mul(
                                out=outt[h * D : (h + 1) * D, qsl],
                                in0=vps_cur[h][:D, cs],
                                in1=rbr[:, h * 128 : (h + 1) * 128],
                            )
                        # both nck projections into one (now free) score tile,
                        # then the two drain copies run on Scalar and Vector
                        # in parallel
                        pp2 = ps_st.tile([128, 2 * QW], F32, tag="st", name="tp")
                        for nck in range(C // 512):
                            nc.tensor.matmul(
                                pp2[:, nck * 512 : (nck + 1) * 512],
                                outt[:, s2 * 128 : (s2 + 1) * 128],
                                wp_sb[:, nck * 512 : (nck + 1) * 512],
                                start=True,
                                stop=True,
                            )
                        for nck in range(C // 512):
                            o_sb = op.tile([128, 512], MMDT, tag="o", name="o_sb")
                            src = pp2[:, nck * 512 : (nck + 1) * 512]
                            if nck == 0:
                                nc.scalar.copy(out=o_sb[:], in_=src)
                            else:
                                nc.vector.tensor_copy(out=o_sb[:], in_=src)
                            nc.sync.dma_start(
                                out_d.ap()[
                                    b0 + s2 * 128 : b0 + (s2 + 1) * 128,
                                    nck * 512 : (nck + 1) * 512,
                                ],
                                o_sb[:],
                            )

            # drain remaining background work
            flush_av()
            while bg:
                pump(1)
            pump_lazy(len(bgl))
    nc.compile()
    return nc


_NC_CACHE = {}


def _get_nc():
    if "nc" not in _NC_CACHE:
        _NC_CACHE["nc"] = build_nc()
    return _NC_CACHE["nc"]


def make_in_maps(x, w_qkv, w_proj):
    np_dt = mybir.dt.np(MMDT)
    x = np.asarray(x, dtype=np.float32)
    w_qkv = np.asarray(w_qkv, dtype=np.float32)
    w_proj = np.asarray(w_proj, dtype=np.float32)
    xt = np.ascontiguousarray(x.reshape(SEQ, C).T.astype(np_dt))
    in_maps = []
    for c in range(NCORES):
        cs = slice(128 * c, 128 * c + 128)
        wslice = np.ascontiguousarray(
            np.concatenate(
                [w_qkv[:, cs], w_qkv[:, C:][:, cs], w_qkv[:, 2 * C :][:, cs]], axis=1
            ).astype(np_dt)
        )
        in_maps.append(
            {
                "xt": xt,
                "wqkv": wslice,
                "wproj": np.ascontiguousarray(w_proj[cs, :].astype(np_dt)),
            }
        )
    return in_maps


def kernel(x, w_qkv, w_proj, b_proj, _run_kwargs=None):
    # snapshot inputs to host numpy before any device/compile interaction
    in_maps = make_in_maps(x, w_qkv, w_proj)
    b_proj = np.asarray(b_proj, dtype=np.float32)
    nc = _get_nc()
    res = run_bass_kernel_spmd(
        nc, in_maps, core_ids=list(range(NCORES)), **(_run_kwargs or {})
    )
    acc = res.results[0]["out"].astype(np.float32)
    for c in range(1, NCORES):
        acc = acc + res.results[c]["out"]
    acc = acc + np.asarray(b_proj, dtype=np.float32)[None, :]
    out = acc.reshape(B, N, C)
    if _run_kwargs:
        kernel.last_result = res
    return out


# revision 23
# speedup vs baseline: 1.0390x; 1.0046x over previous
"""Multi-head attention block (B=2, N=2048, C=1024, H=16) on 8 TRN2 NeuronCores.

Sharding (tensor-parallel over heads): core c owns global heads {2c, 2c+1}:
  - w_qkv columns for q/k/v of those heads  -> [1024, 384] slice
  - w_proj rows for those heads             -> [128, 1024] slice
  - x replicated, pre-transposed on host to xT [1024, 4096] (and cast bf16)
Each core computes a full [4096, 1024] partial of the output projection;
the host sums the 8 partials and adds b_proj.

Device pipeline per core (bf16 matmuls, fp32 PSUM accumulation):
  1. qkvT = w_slice.T @ xT -> qT/kT/vT in [head_dim, seq] layout, emitted
     as single-kt quanta through a clock-budgeted background queue.
  2. Attention per (batch, 512-wide q chunk): both heads' scores^T
     [keys=128, 512] are packed into one [128, 1024] PSUM tile via
     row-group tile_position (the K=64 matmuls run concurrently in the
     PE array), one Exp per chunk on ScalarE (1/sqrt(d) folded into the
     activation scale; no max-subtraction needed for these O(1) scores),
     then a V-matmul per head whose [keys=128, 66] stationary operand is
     [v | ones] - the ones columns make the PSUM accumulator also
     collect softmax denominators.
  3. out^T chunks feed the projection matmul directly as lhsT (k=128,
     no transpose); results stream out per [128, 512] tile.
Scheduling: the emitter runs a static clock model of the PE and ACT
engines. The exp chain is the pacer; V-matmuls are DEFERRED (pt pool
bufs=8 gives ~8 kc of elastic lag) and emitted only when their exp is
predicted complete, with background work (qkv quanta, projection
chunks) pumped into the predicted PE slack so the PE never head-of-line
blocks on the exp. Deadline markers (pump_until) remain as the
correctness net for qkv/vaug availability.
V transposes: batch 0 builds vaug via PE transposes (the DMA ring is
busy streaming xT then); batch 1 uses two whole-batch DMA xbar
transposes on the by-then-quiet ring.
Tail: per-s2 pipeline where the denominator broadcast runs as a tiny
fp32 PE matmul (ones-column outer product) instead of the slow gpsimd
partition_broadcast, so the last out-DMAs leave ~1.5us after the final
V matmul.
"""

import math
import os

import numpy as np

os.environ.setdefault("JAX_PLATFORMS", "axon,cpu")

import concourse.mybir as mybir
import concourse.tile as tile
from concourse import bacc
from concourse.bass_utils import run_bass_kernel_spmd
from concourse.masks import make_identity

F32 = mybir.dt.float32
MMDT = mybir.dt.bfloat16  # matmul operand dtype

# Problem shape (hardcoded per contract)
B, N, C, H = 2, 2048, 1024, 16
D = C // H            # 64 head dim
SEQ = B * N           # 4096
NCORES = 8
HL = H // NCORES      # 2 local heads per core
MW = 3 * HL * D       # 384 w_qkv slice cols (q|k|v for 2 heads)
KT = C // 128         # 8 contraction tiles for the projections
SC = 512              # seq chunk for qkv stage
NSC = SEQ // SC       # 8
KCN = N // 128        # 16 key chunks per batch
QW = 512              # q-chunk width for attention
NQH = N // QW         # 4
SCALE = 1.0 / math.sqrt(D)
PTB = 8               # pt pool depth = max AV lag in kc

# static clock model costs (ns)
C_EXP = 1060          # ScalarE exp of [128, 1024] from PSUM
C_SEM = 150           # cross-engine semaphore latency
C_PAIR = 330          # row-tiled scores pair (LDW + MM 512)
C_AV = 235            # one AV matmul (LDW 66 + MM 512)
C_QKT = 230           # one qkv kt-matmul (FD 512)
C_PROJ = 240          # one proj chunk matmul (FD 512)
C_VAUG0 = 520         # b=0 vaug piece (4 PE transposes)
C_CHEAP = 40          # DVE-only / DMA-only quanta


def build_nc():
    nc = bacc.Bacc("TRN2", target_bir_lowering=False, debug=False)
    xt_d = nc.dram_tensor("xt", [C, SEQ], MMDT, kind="ExternalInput")
    wqkv_d = nc.dram_tensor("wqkv", [C, MW], MMDT, kind="ExternalInput")
    wproj_d = nc.dram_tensor("wproj", [HL * D, C], MMDT, kind="ExternalInput")
    out_d = nc.dram_tensor("out", [SEQ, C], MMDT, kind="ExternalOutput")

    with tile.TileContext(nc) as tc:
        with (
            tc.tile_pool(name="const", bufs=1) as const,
            tc.tile_pool(name="qkvt", bufs=1) as qkvt,
            tc.tile_pool(name="vaugp", bufs=2) as vaugp,
            tc.tile_pool(name="ptp", bufs=PTB) as ptp,
            tc.tile_pool(name="outt", bufs=2) as outtp,
            tc.tile_pool(name="rp", bufs=2) as rp,
            tc.tile_pool(name="op", bufs=5) as op,
            tc.tile_pool(name="ps_st", bufs=2, space="PSUM") as ps_st,
            tc.tile_pool(name="ps_v", bufs=1, space="PSUM") as ps_v,
            tc.tile_pool(name="ps_aux", bufs=2, space="PSUM") as ps_aux,
        ):
            # ---- constants ----
            junk = const.tile([128, 128], MMDT, tag="junk")
            ident = const.tile([128, 128], MMDT, tag="ident")
            ones_sb = const.tile([128, 1], F32, tag="ones")
            ones_row = const.tile([1, D], F32, tag="ones_row")
            w_sb = const.tile([128, KT, MW], MMDT, tag="wqkv")
            wp_sb = const.tile([128, C], MMDT, tag="wproj")
            nc.gpsimd.memset(junk[:], 1.0)
            nc.gpsimd.memset(ones_sb[:], 1.0)
            nc.gpsimd.memset(ones_row[:], 1.0)
            make_identity(nc, ident[:])

            # persistent transposed qkv: [dim-of-2-heads=128, seq]
            q_sb = qkvt.tile([128, SEQ], MMDT, tag="q")
            k_sb = qkvt.tile([128, SEQ], MMDT, tag="k")
            v_sb = qkvt.tile([128, SEQ], MMDT, tag="v")
            dst = [q_sb, k_sb, v_sb]

            # full xT resident; chunk 0 kt-granular (so the first qkv
            # matmuls start as soon as their own kt slice lands) + weights
            # up front, the rest as whole chunks
            xt_all = qkvt.tile([128, KT, SEQ], MMDT, tag="xt_all")

            def xt_dma(sc):
                nc.sync.dma_start(
                    xt_all[:, :, sc * SC : (sc + 1) * SC],
                    xt_d.ap()[:, sc * SC : (sc + 1) * SC].rearrange(
                        "(kt p) n -> p kt n", p=128
                    ),
                )

            # input DMAs in need-order (ring processes doorbells in order):
            # k-weight columns first, then chunk 0 per-kt, q/v weights, proj
            # weights, and the rest
            def xt_dma_kts(sc):
                src = xt_d.ap()[:, sc * SC : (sc + 1) * SC].rearrange(
                    "(kt p) n -> p kt n", p=128
                )
                for kt in range(KT):
                    nc.sync.dma_start(
                        xt_all[:, kt, sc * SC : (sc + 1) * SC], src[:, kt, :]
                    )

            wsrc = wqkv_d.ap().rearrange("(kt p) m -> p kt m", p=128)
            nc.sync.dma_start(w_sb[:, :, 128:256], wsrc[:, :, 128:256])  # k
            xt_dma_kts(0)
            nc.sync.dma_start(w_sb[:, :, 0:128], wsrc[:, :, 0:128])  # q
            xt_dma_kts(1)
            nc.sync.dma_start(w_sb[:, :, 256:384], wsrc[:, :, 256:384])  # v
            xt_dma_kts(2)
            nc.sync.dma_start(wp_sb[:], wproj_d.ap())
            xt_dma_kts(3)
            for sc in range(4, NSC):
                xt_dma(sc)

            # HAM warmup: keep the PE busy during the initial DMA wait so the
            # clock gate is at 8/8 when real work lands (junk operands — no
            # dependency on any DMA). Sized to end roughly when the first
            # xt/weight slices arrive.
            wu = ps_aux.tile([128, SC], F32, tag="aux", name="wu")
            for _ in range(32):
                nc.tensor.matmul(wu[:, :128], junk[:], junk[:], start=True, stop=True)

            # ---- static clock model ----
            CLK = {"pe": 0.0, "act": 0.0}
            exp_end = {}  # global exp index -> predicted completion ns

            # ---- background queue with markers ----
            bg = []   # deadline work: (key|None, cost, closure, opens_group)
            bgl = []  # lazy work (projection chunks): (cost, closure)
            done = set()
            # True while a multi-part qkv PSUM accumulation is mid-flight;
            # lazy proj chunks share the aux tag ring and must not land
            # between two parts of a live accumulation
            grp = {"open": False}

            def run_bg_item():
                key, cost, fn, opens = bg.pop(0)
                fn()
                CLK["pe"] += cost
                grp["open"] = opens
                if key is not None:
                    done.add(key)

            def pump(n=1):
                for _ in range(n):
                    if not bg:
                        return
                    run_bg_item()

            def pump_lazy(n=1):
                for _ in range(n):
                    if not bgl:
                        return
                    cost, fn = bgl.pop(0)
                    fn()
                    CLK["pe"] += cost

            def pump_until(key):
                while key not in done:
                    run_bg_item()

            # ---- qkv + vaug emission (all via bg) ----
            def emit_qkv_part(holder, sc, m, part, nparts=2):
                if part == 0:
                    holder["ps"] = ps_aux.tile([128, SC], F32, tag="aux", name="qkv_ps")
                ps = holder["ps"]
                step = KT // nparts
                for kt in range(part * step, (part + 1) * step):
                    nc.tensor.matmul(
                        ps[:],
                        w_sb[:, kt, m * 128 : (m + 1) * 128],
                        xt_all[:, kt, sc * SC : (sc + 1) * SC],
                        start=(kt == 0),
                        stop=(kt == KT - 1),
                    )
                if part == nparts - 1:
                    # all copies on DVE: the ScalarE queue must stay clear for
                    # the exp chain (sc==0 copies used to sit ahead of exp#0)
                    if sc == 0 and m == 1:
                        # first k block: land the kc=0 key chunk first so the
                        # very first scores matmul can start early
                        nc.vector.tensor_copy(out=dst[m][:, :128], in_=ps[:, :128])
                        nc.vector.tensor_copy(out=dst[m][:, 128:SC], in_=ps[:, 128:])
                    else:
                        nc.vector.tensor_copy(
                            out=dst[m][:, sc * SC : (sc + 1) * SC], in_=ps[:]
                        )

            vaug_store = {}
            vtb_store = {}
            MV = D + 2  # stationary width: v columns + ones (denominator) pad

            vtb_store[0] = vaugp.tile(
                [128, 16, D], MMDT, tag="vtb0", name="vtb", bufs=1
            )
            vtb_store[1] = vaugp.tile(
                [128, 16, D], MMDT, tag="vtb1", name="vtb", bufs=1
            )

            def emit_vtb_dma(h):
                # whole-batch transpose for b=1 on the (by then quiet) DMA
                # ring: vtb[p, T, d] = v[d, N + T*128 + p]
                nc.sync.dma_start_transpose(
                    vtb_store[h][:], v_sb[h * D : (h + 1) * D, N : 2 * N]
                )

            def emit_vaug_tr(holder, b, h, piece):
                b0 = b * N
                va = vaugp.tile([128, 4, MV], MMDT, tag=f"vaug{h}_{piece}", name="va")
                if b == 0 and piece < 3:
                    # JIT path: PE transpose (the DMA ring is busy streaming
                    # xt input during batch 0's first chunk)
                    tr = ps_aux.tile([128, 4 * D], MMDT, tag="aux", name="tr")
                    for tt in range(4):
                        t = piece * 4 + tt
                        nc.tensor.transpose(
                            tr[:, tt * D : (tt + 1) * D],
                            v_sb[h * D : (h + 1) * D, b0 + t * 128 : b0 + (t + 1) * 128],
                            ident[h * D : (h + 1) * D, h * D : (h + 1) * D],
                        )
                    nc.vector.tensor_copy(
                        out=va[:, :, :D], in_=tr[:].rearrange("p (a d) -> p a d", a=4)
                    )
                elif b == 0:
                    # piece 3 is needed late enough (kc12) that its transpose
                    # can ride the ring right after the xt stream drains;
                    # stage through vtb (b=1's later full-tile DMA overwrites)
                    nc.sync.dma_start_transpose(
                        vtb_store[h][:, 12:16, :],
                        v_sb[h * D : (h + 1) * D, b0 + 1536 : b0 + 2048],
                    )
                    nc.vector.tensor_copy(
                        out=va[:, :, :D], in_=vtb_store[h][:, 12:16, :]
                    )
                else:
                    nc.vector.tensor_copy(
                        out=va[:, :, :D],
                        in_=vtb_store[h][:, piece * 4 : (piece + 1) * 4, :],
                    )
                nc.vector.tensor_copy(
                    out=va[:, :, D : D + 2],
                    in_=ones_sb[:, None, :].to_broadcast([128, 4, 2]),
                )
                vaug_store[(b, h, piece)] = va

            def add_qkv(b, m, scl, nparts=2):
                sc = b * (NSC // 2) + scl
                holder = {}
                for part in range(nparts):
                    key = None
                    if part == nparts - 1:
                        key = ("kvq"[0 if m == 1 else 1 if m == 2 else 2], b, scl)
                    bg.append(
                        (
                            key,
                            C_QKT * (KT // nparts),
                            lambda sc=sc, m=m, part=part, holder=holder, np_=nparts: (
                                emit_qkv_part(holder, sc, m, part, np_)
                            ),
                            part < nparts - 1,
                        )
                    )

            def add_vaug(b, scl):
                for h in range(HL):
                    bg.append(
                        (
                            ("vaug", b, h, scl),
                            C_VAUG0 if (b == 0 and scl < 3) else C_CHEAP,
                            lambda b=b, h=h, scl=scl: emit_vaug_tr({}, b, h, scl),
                            False,
                        )
                    )

            # b=0: block 0 ordered for the fastest first exp (k, q, then v);
            # k-blocks early (scores need them at kc=4*blk), q for the next
            # chunk pulled ahead of late v-blocks so chunk transitions don't
            # burst
            add_qkv(0, 1, 0)
            add_qkv(0, 0, 0)
            add_qkv(0, 1, 1)
            add_qkv(0, 2, 0)
            add_vaug(0, 0)
            add_qkv(0, 1, 2)
            add_qkv(0, 2, 1)
            add_vaug(0, 1)
            add_qkv(0, 1, 3)
            add_qkv(0, 2, 2)
            add_vaug(0, 2)
            add_qkv(0, 0, 1)
            add_qkv(0, 2, 3)
            add_vaug(0, 3)
            add_qkv(0, 0, 2)
            add_qkv(0, 0, 3)
            # b=1: k/v first, whole-batch vaug transposes ride the (by now
            # quiet) DMA ring, q spread between
            add_qkv(1, 1, 0)
            add_qkv(1, 0, 0)
            add_qkv(1, 2, 0)
            add_qkv(1, 1, 1)
            add_qkv(1, 2, 1)
            add_qkv(1, 1, 2)
            add_qkv(1, 2, 2)
            add_qkv(1, 0, 1)
            add_qkv(1, 1, 3)
            add_qkv(1, 2, 3)
            # vtb DMA transposes read ALL of b=1's v — they must come after
            # every b=1 v-block is queued (the tracker cannot wait on writes
            # emitted later in program order)
            bg.append((None, C_CHEAP, lambda: emit_vtb_dma(0), False))
            bg.append((None, C_CHEAP, lambda: emit_vtb_dma(1), False))
            add_vaug(1, 0)
            add_vaug(1, 1)
            add_qkv(1, 0, 2)
            add_vaug(1, 2)
            add_vaug(1, 3)
            add_qkv(1, 0, 3)

            def emit_proj_chunk(outt, b0, s2, nck):
                pp = ps_aux.tile([128, 512], F32, tag="aux", name="proj_ps")
                nc.tensor.matmul(
                    pp[:],
                    outt[:, s2 * 128 : (s2 + 1) * 128],
                    wp_sb[:, nck * 512 : (nck + 1) * 512],
                    start=True,
                    stop=True,
                )
                o_sb = op.tile([128, 512], MMDT, tag="o", name="o_sb")
                nc.vector.tensor_copy(out=o_sb[:], in_=pp[:])
                nc.sync.dma_start(
                    out_d.ap()[
                        b0 + s2 * 128 : b0 + (s2 + 1) * 128,
                        nck * 512 : (nck + 1) * 512,
                    ],
                    o_sb[:],
                )

            # ---- attention ----
            g_scores = {}  # global kc index -> predicted scores-done ns

            def emit_scores(b, qh, kc, g):
                b0 = b * N
                q0 = b0 + qh * QW
                pump_until(("q", b, qh))
                pump_until(("k", b, kc // 4))
                st = ps_st.tile([128, 2 * QW], F32, tag="st", name="st")
                # top priority: the exp chain paces the kernel; splitting the
                # row-packed pair stalls the exp at chunk boundaries
                with tc.high_priority():
                    for h in range(HL):
                        hs = slice(h * D, (h + 1) * D)
                        nc.tensor.matmul(
                            st[:, h * QW : (h + 1) * QW],
                            k_sb[hs, b0 + kc * 128 : b0 + (kc + 1) * 128],
                            q_sb[hs, q0 : q0 + QW],
                            start=True,
                            stop=True,
                            tile_position=(h * D, 0),
                        )
                # st WAR: this buffer was read by exp g-2
                CLK["pe"] = max(CLK["pe"], exp_end.get(g - 2, 0.0)) + C_PAIR
                g_scores[g] = CLK["pe"]
                return st

            # deferred AV queue: entries emitted only when their exp is
            # predicted done (or forced by pt-pool WAR / chunk drain)
            pend_av = []
            av_done = {}  # global exp index -> PE completion of its AVs

            def emit_av(e):
                b, kc = e["b"], e["kc"]
                for h in range(HL):
                    pump_until(("vaug", b, h, kc // 4))
                for h in range(HL):
                    nc.tensor.matmul(
                        e["vps"][h][:MV, :],
                        vaug_store[(b, h, kc // 4)][:, kc % 4, :],
                        e["pt"][:, h * QW : (h + 1) * QW],
                        start=(kc == 0),
                        stop=(kc == KCN - 1),
                    )
                CLK["pe"] = max(CLK["pe"], exp_end[e["g"]]) + 2 * C_AV
                av_done[e["g"]] = CLK["pe"]
                if e["fin"] is not None:
                    e["fin"]()

            def flush_av(n=None):
                cnt = len(pend_av) if n is None else n
                for _ in range(cnt):
                    if not pend_av:
                        return
                    emit_av(pend_av.pop(0))

            chunks = [(b, qh) for b in range(B) for qh in range(NQH)]
            pend = emit_scores(0, 0, 0, 0)
            outt = outu = None
            rs2 = None
            vps_cur = None
            for ci, (b, qh) in enumerate(chunks):
                b0 = b * N
                last = ci == len(chunks) - 1
                if qh == 0:
                    outt = outtp.tile([128, N], MMDT, tag="outT", name="outt")
                    outu = [
                        outtp.tile([D, N], MMDT, tag=f"outu{h}", name="outu")
                        for h in range(HL)
                    ]
                    # [1, qh, h, q] so each chunk's denominator block is
                    # contiguous (2D-viewable for partition_broadcast)
                    rs2 = rp.tile([1, NQH, HL, QW], F32, tag="rs2", name="rs2")
                vps_cur = [
                    ps_v.tile([128, QW], F32, tag=f"vps{h}", name=f"vps{h}")
                    for h in range(HL)
                ]
                qs = slice(qh * QW, (qh + 1) * QW)

                # per-chunk finalizer, attached to the last AV of the chunk:
                # drain vps to SBUF, normalize off the critical path, queue
                # this q-range's projection chunks as lazy work
                def make_fin(
                    b=b, qh=qh, b0=b0, outt=outt, outu=outu, rs2=rs2,
                    vps=vps_cur, qs=qs, last=last,
                ):
                    def fin():
                        if last:
                            return  # tail handled inline below
                        for h in range(HL):
                            nc.vector.tensor_copy(out=outu[h][:, qs], in_=vps[h][:D, :])
                            nc.vector.tensor_copy(
                                out=rs2[:, qh, h, :], in_=vps[h][D : D + 1, :]
                            )
                        # one broadcast+recip covers both heads' denominators
                        rb = rp.tile([D, HL * QW], F32, tag="rb", name="rb")
                        nc.gpsimd.partition_broadcast(
                            rb[:], rs2[:, qh].rearrange("p h q -> p (h q)")
                        )
                        rbr = rp.tile([D, HL * QW], F32, tag="rbr", name="rbr")
                        nc.vector.reciprocal_approx_fast(out=rbr[:], in_=rb[:])
                        for h in range(HL):
                            nc.vector.tensor_mul(
                                out=outt[h * D : (h + 1) * D, qs],
                                in0=outu[h][:, qs],
                                in1=rbr[:, h * QW : (h + 1) * QW],
                            )
                        for s2 in range(qh * (QW // 128), (qh + 1) * (QW // 128)):
                            for nck in range(C // 512):
                                bgl.append(
                                    (
                                        C_PROJ,
                                        lambda outt=outt, b0=b0, s2=s2, nck=nck: (
                                            emit_proj_chunk(outt, b0, s2, nck)
                                        ),
                                    )
                                )
                    return fin

                fin_cb = make_fin()
                for kc in range(KCN):
                    g = ci * KCN + kc
                    if kc + 1 < KCN:
                        nxt = emit_scores(b, qh, kc + 1, g + 1)
                    elif ci + 1 < len(chunks):
                        nb, nqh = chunks[ci + 1]
                        nxt = emit_scores(nb, nqh, 0, g + 1)
                    else:
                        nxt = None

                    # pt-pool WAR: the buffer exp(g) writes was read by the
                    # AVs of exp g-PTB+1; those must be emitted first
                    while pend_av and pend_av[0]["g"] <= g - (PTB - 1):
                        emit_av(pend_av.pop(0))

                    pt = ptp.tile([128, 2 * QW], MMDT, tag="pt")
                    nc.scalar.activation(
                        out=pt[:],
                        in_=pend[:],
                        func=mybir.ActivationFunctionType.Exp,
                        scale=SCALE,
                    )
                    e_end = (
                        max(
                            CLK["act"],
                            g_scores[g] + C_SEM,
                            av_done.get(g - PTB, 0.0) + C_SEM,
                        )
                        + C_EXP
                    )
                    CLK["act"] = e_end
                    exp_end[g] = e_end
                    pend_av.append(
                        {
                            "b": b, "kc": kc, "g": g, "pt": pt,
                            "vps": vps_cur, "fin": fin_cb if kc == KCN - 1 else None,
                        }
                    )

                    # first drain every ripe AV (its exp is comfortably done —
                    # the PE never stalls on these and they gate the exp
                    # chain via the pt-pool WAR horizon)
                    while pend_av and exp_end[pend_av[0]["g"]] <= CLK["pe"] - 150.0:
                        emit_av(pend_av.pop(0))

                    # credit-based background pumping: spread the remaining
                    # queue cost evenly over the remaining kcs, independent of
                    # the PE-vs-ACT clock drift (the scores pairs are
                    # priority-hoisted, so queueing background early cannot
                    # delay the exp chain; the PE FIFO just stays fed).
                    # Lazy proj chunks run only once the qkv queue is empty —
                    # they share the aux PSUM ring with the qkv accumulators.
                    rem_kc = max(len(chunks) * KCN - 1 - g, 1)
                    rem_cost = sum(it[1] for it in bg) + sum(c for c, _ in bgl)
                    credit = rem_cost / rem_kc
                    if ci == len(chunks) - 1:
                        credit = rem_cost
                    while credit > 0.0:
                        if pend_av and exp_end[pend_av[0]["g"]] <= CLK["pe"] - 150.0:
                            emit_av(pend_av.pop(0))
                        elif bg:
                            credit -= bg[0][1]
                            run_bg_item()
                        elif bgl:
                            credit -= bgl[0][0]
                            pump_lazy(1)
                        else:
                            break

                    pend = nxt

                if last:
                    # flush every remaining AV (and any leftover background)
                    flush_av()
                    while bg:
                        pump(1)
                    pump_lazy(len(bgl))
                    # tail: fine-grained per-s2 pipeline — PE ones-column
                    # matmul broadcasts the two denominators (fp32, ~0.4us
                    # cheaper than gpsimd partition_broadcast), recip, two
                    # muls, then this s2's projections and out-DMAs
                    nc.scalar.copy(out=rs2[:, qh, 0, :], in_=vps_cur[0][D : D + 1, :])
                    nc.vector.tensor_copy(
                        out=rs2[:, qh, 1, :], in_=vps_cur[1][D : D + 1, :]
                    )
                    for s2l in range(QW // 128):
                        s2 = qh * (QW // 128) + s2l
                        cs = slice(s2l * 128, (s2l + 1) * 128)
                        qsl = slice(
                            qh * QW + s2l * 128, qh * QW + (s2l + 1) * 128
                        )
                        rbp = ps_aux.tile([128, SC], F32, tag="aux", name="rbp")
                        nc.tensor.matmul(
                            rbp[:D, : HL * 128],
                            ones_row[:, :],
                            rs2[:, qh, :, s2l * 128 : (s2l + 1) * 128],
                            start=True,
                            stop=True,
                        )
                        rbr = rp.tile([D, HL * 128], F32, tag="rbrs", name="rbrs")
                        nc.vector.reciprocal_approx_fast(
                            out=rbr[:], in_=rbp[:D, : HL * 128]
                        )
                        for h in range(HL):
                            nc.vector.tensor_mul(
                                out=outt[h * D : (h + 1) * D, qsl],
                                in0=vps_cur[h][:D, cs],
                                in1=rbr[:, h * 128 : (h + 1) * 128],
                            )
                        # both nck projections into one (now free) score tile,
                        # then the two drain copies run on Scalar and Vector
                        # in parallel
                        pp2 = ps_st.tile([128, 2 * QW], F32, tag="st", name="tp")
                        for nck in range(C // 512):
                            nc.tensor.matmul(
                                pp2[:, nck * 512 : (nck + 1) * 512],
                                outt[:, s2 * 128 : (s2 + 1) * 128],
                                wp_sb[:, nck * 512 : (nck + 1) * 512],
                                start=True,
                                stop=True,
                            )
                        for nck in range(C // 512):
                            o_sb = op.tile([128, 512], MMDT, tag="o", name="o_sb")
                            src = pp2[:, nck * 512 : (nck + 1) * 512]
                            if nck == 0:
                                nc.scalar.copy(out=o_sb[:], in_=src)
                            else:
                                nc.vector.tensor_copy(out=o_sb[:], in_=src)
                            nc.sync.dma_start(
                                out_d.ap()[
                                    b0 + s2 * 128 : b0 + (s2 + 1) * 128,
                                    nck * 512 : (nck + 1) * 512,
                                ],
                                o_sb[:],
                            )

            # drain remaining background work
            flush_av()
            while bg:
                pump(1)
            pump_lazy(len(bgl))
    nc.compile()
    return nc


_NC_CACHE = {}


def _get_nc():
    if "nc" not in _NC_CACHE:
        _NC_CACHE["nc"] = build_nc()
    return _NC_CACHE["nc"]


def make_in_maps(x, w_qkv, w_proj):
    np_dt = mybir.dt.np(MMDT)
    x = np.asarray(x, dtype=np.float32)
    w_qkv = np.asarray(w_qkv, dtype=np.float32)
    w_proj = np.asarray(w_proj, dtype=np.float32)
    xt = np.ascontiguousarray(x.reshape(SEQ, C).T.astype(np_dt))
    in_maps = []
    for c in range(NCORES):
        cs = slice(128 * c, 128 * c + 128)
        wslice = np.ascontiguousarray(
            np.concatenate(
                [w_qkv[:, cs], w_qkv[:, C:][:, cs], w_qkv[:, 2 * C :][:, cs]], axis=1
            ).astype(np_dt)
        )
        in_maps.append(
            {
                "xt": xt,
                "wqkv": wslice,
                "wproj": np.ascontiguousarray(w_proj[cs, :].astype(np_dt)),
            }
        )
    return in_maps


def kernel(x, w_qkv, w_proj, b_proj, _run_kwargs=None):
    # snapshot inputs to host numpy before any device/compile interaction
    in_maps = make_in_maps(x, w_qkv, w_proj)
    b_proj = np.asarray(b_proj, dtype=np.float32)
    nc = _get_nc()
    res = run_bass_kernel_spmd(
        nc, in_maps, core_ids=list(range(NCORES)), **(_run_kwargs or {})
    )
    acc = res.results[0]["out"].astype(np.float32)
    for c in range(1, NCORES):
        acc = acc + res.results[c]["out"]
    acc = acc + np.asarray(b_proj, dtype=np.float32)[None, :]
    out = acc.reshape(B, N, C)
    if _run_kwargs:
        kernel.last_result = res
    return out


# revision 26
# speedup vs baseline: 1.0478x; 1.0085x over previous
"""Multi-head attention block (B=2, N=2048, C=1024, H=16) on 8 TRN2 NeuronCores.

Sharding (tensor-parallel over heads): core c owns global heads {2c, 2c+1}:
  - w_qkv columns for q/k/v of those heads  -> [1024, 384] slice
  - w_proj rows for those heads             -> [128, 1024] slice
  - x replicated, pre-transposed on host to xT [1024, 4096] (and cast bf16)
Each core computes a full [4096, 1024] partial of the output projection;
the host sums the 8 partials and adds b_proj.

Device pipeline per core (bf16 matmuls, fp32 PSUM accumulation):
  1. qkvT = w_slice.T @ xT -> qT/kT/vT in [head_dim, seq] layout, emitted
     as single-kt quanta through a clock-budgeted background queue.
  2. Attention per (batch, 512-wide q chunk): both heads' scores^T
     [keys=128, 512] are packed into one [128, 1024] PSUM tile via
     row-group tile_position (the K=64 matmuls run concurrently in the
     PE array), one Exp per chunk on ScalarE (1/sqrt(d) folded into the
     activation scale; no max-subtraction needed for these O(1) scores),
     then a V-matmul per head whose [keys=128, 66] stationary operand is
     [v | ones] - the ones columns make the PSUM accumulator also
     collect softmax denominators.
  3. out^T chunks feed the projection matmul directly as lhsT (k=128,
     no transpose); results stream out per [128, 512] tile.
Scheduling: the emitter runs a static clock model of the PE and ACT
engines. The exp chain is the pacer; V-matmuls are DEFERRED (pt pool
bufs=8 gives ~8 kc of elastic lag) and emitted only when their exp is
predicted complete, with background work (qkv quanta, projection
chunks) pumped into the predicted PE slack so the PE never head-of-line
blocks on the exp. Deadline markers (pump_until) remain as the
correctness net for qkv/vaug availability.
V transposes: batch 0 builds vaug via PE transposes (the DMA ring is
busy streaming xT then); batch 1 uses two whole-batch DMA xbar
transposes on the by-then-quiet ring.
Tail: per-s2 pipeline where the denominator broadcast runs as a tiny
fp32 PE matmul (ones-column outer product) instead of the slow gpsimd
partition_broadcast, so the last out-DMAs leave ~1.5us after the final
V matmul.
"""

import math
import os

import numpy as np

os.environ.setdefault("JAX_PLATFORMS", "axon,cpu")

import concourse.mybir as mybir
import concourse.tile as tile
from concourse import bacc
from concourse.bass_utils import run_bass_kernel_spmd
from concourse.masks import make_identity

F32 = mybir.dt.float32
MMDT = mybir.dt.bfloat16  # matmul operand dtype

# Problem shape (hardcoded per contract)
B, N, C, H = 2, 2048, 1024, 16
D = C // H            # 64 head dim
SEQ = B * N           # 4096
NCORES = 8
HL = H // NCORES      # 2 local heads per core
MW = 3 * HL * D       # 384 w_qkv slice cols (q|k|v for 2 heads)
KT = C // 128         # 8 contraction tiles for the projections
SC = 512              # seq chunk for qkv stage
NSC = SEQ // SC       # 8
KCN = N // 128        # 16 key chunks per batch
QW = 512              # q-chunk width for attention
NQH = N // QW         # 4
SCALE = 1.0 / math.sqrt(D)
PTB = 8               # pt pool depth = max AV lag in kc

# static clock model costs (ns)
C_EXP = 1060          # ScalarE exp of [128, 1024] from PSUM
C_SEM = 150           # cross-engine semaphore latency
C_PAIR = 330          # row-tiled scores pair (LDW + MM 512)
C_AV = 235            # one AV matmul (LDW 66 + MM 512)
C_QKT = 230           # one qkv kt-matmul (FD 512)
C_PROJ = 240          # one proj chunk matmul (FD 512)
C_VAUG0 = 520         # b=0 vaug piece (4 PE transposes)
C_CHEAP = 40          # DVE-only / DMA-only quanta


def build_nc():
    nc = bacc.Bacc("TRN2", target_bir_lowering=False, debug=False)
    xt_d = nc.dram_tensor("xt", [C, SEQ], MMDT, kind="ExternalInput")
    wqkv_d = nc.dram_tensor("wqkv", [C, MW], MMDT, kind="ExternalInput")
    wproj_d = nc.dram_tensor("wproj", [HL * D, C], MMDT, kind="ExternalInput")
    out_d = nc.dram_tensor("out", [SEQ, C], MMDT, kind="ExternalOutput")

    with tile.TileContext(nc) as tc:
        with (
            tc.tile_pool(name="const", bufs=1) as const,
            tc.tile_pool(name="qkvt", bufs=1) as qkvt,
            tc.tile_pool(name="vaugp", bufs=2) as vaugp,
            tc.tile_pool(name="ptp", bufs=PTB) as ptp,
            tc.tile_pool(name="outt", bufs=2) as outtp,
            tc.tile_pool(name="rp", bufs=2) as rp,
            tc.tile_pool(name="op", bufs=5) as op,
            tc.tile_pool(name="ps_st", bufs=2, space="PSUM") as ps_st,
            tc.tile_pool(name="ps_v", bufs=1, space="PSUM") as ps_v,
            tc.tile_pool(name="ps_aux", bufs=2, space="PSUM") as ps_aux,
        ):
            # ---- constants ----
            junk = const.tile([128, 128], MMDT, tag="junk")
            ident = const.tile([128, 128], MMDT, tag="ident")
            ones_sb = const.tile([128, 1], F32, tag="ones")
            ones_row = const.tile([1, D], F32, tag="ones_row")
            w_sb = const.tile([128, KT, MW], MMDT, tag="wqkv")
            wp_sb = const.tile([128, C], MMDT, tag="wproj")
            tld = const.tile([1, 1], F32, tag="tld")
            nc.gpsimd.memset(junk[:], 1.0)
            nc.gpsimd.memset(ones_sb[:], 1.0)
            nc.gpsimd.memset(ones_row[:], 1.0)
            make_identity(nc, ident[:])
            # pre-trigger the exp ACT table load (~2.7us) during the initial
            # DMA wait so exp#0 doesn't pay it
            nc.scalar.activation(
                out=tld[:], in_=ones_sb[:1, :],
                func=mybir.ActivationFunctionType.Exp,
            )

            # persistent transposed qkv: [dim-of-2-heads=128, seq]
            q_sb = qkvt.tile([128, SEQ], MMDT, tag="q")
            k_sb = qkvt.tile([128, SEQ], MMDT, tag="k")
            v_sb = qkvt.tile([128, SEQ], MMDT, tag="v")
            dst = [q_sb, k_sb, v_sb]

            # full xT resident; chunk 0 kt-granular (so the first qkv
            # matmuls start as soon as their own kt slice lands) + weights
            # up front, the rest as whole chunks
            xt_all = qkvt.tile([128, KT, SEQ], MMDT, tag="xt_all")

            def xt_dma(sc):
                nc.sync.dma_start(
                    xt_all[:, :, sc * SC : (sc + 1) * SC],
                    xt_d.ap()[:, sc * SC : (sc + 1) * SC].rearrange(
                        "(kt p) n -> p kt n", p=128
                    ),
                )

            # input DMAs in need-order (ring processes doorbells in order):
            # k-weight columns first, then chunk 0 per-kt, q/v weights, proj
            # weights, and the rest
            def xt_dma_kts(sc):
                src = xt_d.ap()[:, sc * SC : (sc + 1) * SC].rearrange(
                    "(kt p) n -> p kt n", p=128
                )
                for kt in range(KT):
                    nc.sync.dma_start(
                        xt_all[:, kt, sc * SC : (sc + 1) * SC], src[:, kt, :]
                    )

            # w_qkv arrives host-pre-permuted to [p, kt, m] order so the whole
            # weight block is one 128 x 6KB-descriptor transfer (the old
            # (kt p) m layout generated 3072 tiny 256B descriptors)
            wsrc = wqkv_d.ap().rearrange("(p kt) m -> p kt m", p=128)
            nc.sync.dma_start(w_sb[:], wsrc)
            xt_dma_kts(0)
            xt_dma_kts(1)
            nc.sync.dma_start(wp_sb[:], wproj_d.ap())
            xt_dma_kts(2)
            xt_dma_kts(3)
            # b1 chunks paired: [128, kt, 1024] runs = 2KB descriptors
            for sc2 in (2, 3):
                nc.sync.dma_start(
                    xt_all[:, :, sc2 * 2 * SC : (sc2 + 1) * 2 * SC],
                    xt_d.ap()[:, sc2 * 2 * SC : (sc2 + 1) * 2 * SC].rearrange(
                        "(kt p) n -> p kt n", p=128
                    ),
                )

            # HAM warmup: keep the PE busy during the initial DMA wait so the
            # clock gate is at 8/8 when real work lands (junk operands — no
            # dependency on any DMA). Sized to end roughly when the first
            # xt/weight slices arrive.
            wu = ps_aux.tile([128, SC], F32, tag="aux", name="wu")
            for _ in range(32):
                nc.tensor.matmul(wu[:, :128], junk[:], junk[:], start=True, stop=True)

            # ---- static clock model ----
            CLK = {"pe": 0.0, "act": 0.0}
            exp_end = {}  # global exp index -> predicted completion ns

            # ---- background queue with markers ----
            bg = []   # deadline work: (key|None, cost, closure, opens_group)
            bgl = []  # lazy work (projection chunks): (cost, closure)
            done = set()
            # True while a multi-part qkv PSUM accumulation is mid-flight;
            # lazy proj chunks share the aux tag ring and must not land
            # between two parts of a live accumulation
            grp = {"open": False}

            def run_bg_item():
                key, cost, fn, opens = bg.pop(0)
                fn()
                CLK["pe"] += cost
                grp["open"] = opens
                if key is not None:
                    done.add(key)

            def pump(n=1):
                for _ in range(n):
                    if not bg:
                        return
                    run_bg_item()

            def pump_lazy(n=1):
                for _ in range(n):
                    if not bgl:
                        return
                    cost, fn = bgl.pop(0)
                    fn()
                    CLK["pe"] += cost

            def pump_until(key):
                while key not in done:
                    run_bg_item()

            # ---- qkv + vaug emission (all via bg) ----
            def emit_qkv_part(holder, sc, m, part, nparts=2):
                if part == 0:
                    holder["ps"] = ps_aux.tile([128, SC], F32, tag="aux", name="qkv_ps")
                ps = holder["ps"]
                step = KT // nparts
                for kt in range(part * step, (part + 1) * step):
                    nc.tensor.matmul(
                        ps[:],
                        w_sb[:, kt, m * 128 : (m + 1) * 128],
                        xt_all[:, kt, sc * SC : (sc + 1) * SC],
                        start=(kt == 0),
                        stop=(kt == KT - 1),
                    )
                if part == nparts - 1:
                    # all copies on DVE: the ScalarE queue must stay clear for
                    # the exp chain (sc==0 copies used to sit ahead of exp#0)
                    if sc == 0 and m == 1:
                        # first k block: land the kc=0 key chunk first so the
                        # very first scores matmul can start early
                        nc.vector.tensor_copy(out=dst[m][:, :128], in_=ps[:, :128])
                        nc.vector.tensor_copy(out=dst[m][:, 128:SC], in_=ps[:, 128:])
                    else:
                        nc.vector.tensor_copy(
                            out=dst[m][:, sc * SC : (sc + 1) * SC], in_=ps[:]
                        )

            vaug_store = {}
            vtb_store = {}
            MV = D + 2  # stationary width: v columns + ones (denominator) pad

            vtb_store[0] = vaugp.tile(
                [128, 16, D], MMDT, tag="vtb0", name="vtb", bufs=1
            )
            vtb_store[1] = vaugp.tile(
                [128, 16, D], MMDT, tag="vtb1", name="vtb", bufs=1
            )

            def emit_vtb_dma(h):
                # whole-batch transpose for b=1 on the (by then quiet) DMA
                # ring: vtb[p, T, d] = v[d, N + T*128 + p]
                nc.sync.dma_start_transpose(
                    vtb_store[h][:], v_sb[h * D : (h + 1) * D, N : 2 * N]
                )

            def emit_vaug_tr(holder, b, h, piece):
                b0 = b * N
                va = vaugp.tile([128, 4, MV], MMDT, tag=f"vaug{h}_{piece}", name="va")
                if b == 0 and piece < 3:
                    # JIT path: PE transpose (the DMA ring is busy streaming
                    # xt input during batch 0's first chunk)
                    tr = ps_aux.tile([128, 4 * D], MMDT, tag="aux", name="tr")
                    for tt in range(4):
                        t = piece * 4 + tt
                        nc.tensor.transpose(
                            tr[:, tt * D : (tt + 1) * D],
                            v_sb[h * D : (h + 1) * D, b0 + t * 128 : b0 + (t + 1) * 128],
                            ident[h * D : (h + 1) * D, h * D : (h + 1) * D],
                        )
                    nc.vector.tensor_copy(
                        out=va[:, :, :D], in_=tr[:].rearrange("p (a d) -> p a d", a=4)
                    )
                elif b == 0:
                    # piece 3 is needed late enough (kc12) that its transpose
                    # can ride the ring right after the xt stream drains;
                    # stage through vtb (b=1's later full-tile DMA overwrites)
                    nc.sync.dma_start_transpose(
                        vtb_store[h][:, 12:16, :],
                        v_sb[h * D : (h + 1) * D, b0 + 1536 : b0 + 2048],
                    )
                    nc.vector.tensor_copy(
                        out=va[:, :, :D], in_=vtb_store[h][:, 12:16, :]
                    )
                else:
                    nc.vector.tensor_copy(
                        out=va[:, :, :D],
                        in_=vtb_store[h][:, piece * 4 : (piece + 1) * 4, :],
                    )
                nc.vector.tensor_copy(
                    out=va[:, :, D : D + 2],
                    in_=ones_sb[:, None, :].to_broadcast([128, 4, 2]),
                )
                vaug_store[(b, h, piece)] = va

            def add_qkv(b, m, scl, nparts=2):
                sc = b * (NSC // 2) + scl
                holder = {}
                for part in range(nparts):
                    key = None
                    if part == nparts - 1:
                        key = ("kvq"[0 if m == 1 else 1 if m == 2 else 2], b, scl)
                    bg.append(
                        (
                            key,
                            C_QKT * (KT // nparts),
                            lambda sc=sc, m=m, part=part, holder=holder, np_=nparts: (
                                emit_qkv_part(holder, sc, m, part, np_)
                            ),
                            part < nparts - 1,
                        )
                    )

            def add_vaug(b, scl):
                for h in range(HL):
                    bg.append(
                        (
                            ("vaug", b, h, scl),
                            C_VAUG0 if (b == 0 and scl < 3) else C_CHEAP,
                            lambda b=b, h=h, scl=scl: emit_vaug_tr({}, b, h, scl),
                            False,
                        )
                    )

            # b=0: block 0 ordered for the fastest first exp (k, q, then v);
            # k-blocks early (scores need them at kc=4*blk), q for the next
            # chunk pulled ahead of late v-blocks so chunk transitions don't
            # burst
            add_qkv(0, 1, 0)
            add_qkv(0, 0, 0)
            add_qkv(0, 1, 1)
            add_qkv(0, 2, 0)
            add_vaug(0, 0)
            add_qkv(0, 1, 2)
            add_qkv(0, 2, 1)
            add_vaug(0, 1)
            add_qkv(0, 1, 3)
            add_qkv(0, 2, 2)
            add_vaug(0, 2)
            add_qkv(0, 0, 1)
            add_qkv(0, 2, 3)
            add_vaug(0, 3)
            add_qkv(0, 0, 2)
            add_qkv(0, 0, 3)
            # b=1: k/v first, whole-batch vaug transposes ride the (by now
            # quiet) DMA ring, q spread between
            add_qkv(1, 1, 0)
            add_qkv(1, 0, 0)
            add_qkv(1, 2, 0)
            add_qkv(1, 1, 1)
            add_qkv(1, 2, 1)
            add_qkv(1, 1, 2)
            add_qkv(1, 2, 2)
            add_qkv(1, 0, 1)
            add_qkv(1, 1, 3)
            add_qkv(1, 2, 3)
            # vtb DMA transposes read ALL of b=1's v — they must come after
            # every b=1 v-block is queued (the tracker cannot wait on writes
            # emitted later in program order)
            bg.append((None, C_CHEAP, lambda: emit_vtb_dma(0), False))
            bg.append((None, C_CHEAP, lambda: emit_vtb_dma(1), False))
            add_vaug(1, 0)
            add_vaug(1, 1)
            add_qkv(1, 0, 2)
            add_vaug(1, 2)
            add_vaug(1, 3)
            add_qkv(1, 0, 3)

            def emit_proj_chunk(outt, b0, s2, nck):
                pp = ps_aux.tile([128, 512], F32, tag="aux", name="proj_ps")
                nc.tensor.matmul(
                    pp[:],
                    outt[:, s2 * 128 : (s2 + 1) * 128],
                    wp_sb[:, nck * 512 : (nck + 1) * 512],
                    start=True,
                    stop=True,
                )
                o_sb = op.tile([128, 512], MMDT, tag="o", name="o_sb")
                nc.vector.tensor_copy(out=o_sb[:], in_=pp[:])
                nc.sync.dma_start(
                    out_d.ap()[
                        b0 + s2 * 128 : b0 + (s2 + 1) * 128,
                        nck * 512 : (nck + 1) * 512,
                    ],
                    o_sb[:],
                )

            # ---- attention ----
            g_scores = {}  # global kc index -> predicted scores-done ns

            def emit_scores(b, qh, kc, g):
                b0 = b * N
                q0 = b0 + qh * QW
                pump_until(("q", b, qh))
                pump_until(("k", b, kc // 4))
                st = ps_st.tile([128, 2 * QW], F32, tag="st", name="st")
                # top priority: the exp chain paces the kernel; splitting the
                # row-packed pair stalls the exp at chunk boundaries
                with tc.high_priority():
                    for h in range(HL):
                        hs = slice(h * D, (h + 1) * D)
                        nc.tensor.matmul(
                            st[:, h * QW : (h + 1) * QW],
                            k_sb[hs, b0 + kc * 128 : b0 + (kc + 1) * 128],
                            q_sb[hs, q0 : q0 + QW],
                            start=True,
                            stop=True,
                            tile_position=(h * D, 0),
                        )
                # st WAR: this buffer was read by exp g-2
                CLK["pe"] = max(CLK["pe"], exp_end.get(g - 2, 0.0)) + C_PAIR
                g_scores[g] = CLK["pe"]
                return st

            # deferred AV queue: entries emitted only when their exp is
            # predicted done (or forced by pt-pool WAR / chunk drain)
            pend_av = []
            av_done = {}  # global exp index -> PE completion of its AVs

            def emit_av(e):
                b, kc = e["b"], e["kc"]
                for h in range(HL):
                    pump_until(("vaug", b, h, kc // 4))
                for h in range(HL):
                    nc.tensor.matmul(
                        e["vps"][h][:MV, :],
                        vaug_store[(b, h, kc // 4)][:, kc % 4, :],
                        e["pt"][:, h * QW : (h + 1) * QW],
                        start=(kc == 0),
                        stop=(kc == KCN - 1),
                    )
                CLK["pe"] = max(CLK["pe"], exp_end[e["g"]]) + 2 * C_AV
                av_done[e["g"]] = CLK["pe"]
                if e["fin"] is not None:
                    e["fin"]()

            def flush_av(n=None):
                cnt = len(pend_av) if n is None else n
                for _ in range(cnt):
                    if not pend_av:
                        return
                    emit_av(pend_av.pop(0))

            chunks = [(b, qh) for b in range(B) for qh in range(NQH)]
            pend = emit_scores(0, 0, 0, 0)
            outt = outu = None
            rs2 = None
            vps_cur = None
            for ci, (b, qh) in enumerate(chunks):
                b0 = b * N
                last = ci == len(chunks) - 1
                if qh == 0:
                    outt = outtp.tile([128, N], MMDT, tag="outT", name="outt")
                    outu = [
                        outtp.tile([D, N], MMDT, tag=f"outu{h}", name="outu")
                        for h in range(HL)
                    ]
                    # [1, qh, h, q] so each chunk's denominator block is
                    # contiguous (2D-viewable for partition_broadcast)
                    rs2 = rp.tile([1, NQH, HL, QW], F32, tag="rs2", name="rs2")
                vps_cur = [
                    ps_v.tile([128, QW], F32, tag=f"vps{h}", name=f"vps{h}")
                    for h in range(HL)
                ]
                qs = slice(qh * QW, (qh + 1) * QW)

                # per-chunk finalizer, attached to the last AV of the chunk:
                # drain vps to SBUF, normalize off the critical path, queue
                # this q-range's projection chunks as lazy work
                def make_fin(
                    b=b, qh=qh, b0=b0, outt=outt, outu=outu, rs2=rs2,
                    vps=vps_cur, qs=qs, last=last,
                ):
                    def fin():
                        if last:
                            return  # tail handled inline below
                        for h in range(HL):
                            nc.vector.tensor_copy(out=outu[h][:, qs], in_=vps[h][:D, :])
                            nc.vector.tensor_copy(
                                out=rs2[:, qh, h, :], in_=vps[h][D : D + 1, :]
                            )
                        # one broadcast+recip covers both heads' denominators
                        rb = rp.tile([D, HL * QW], F32, tag="rb", name="rb")
                        nc.gpsimd.partition_broadcast(
                            rb[:], rs2[:, qh].rearrange("p h q -> p (h q)")
                        )
                        rbr = rp.tile([D, HL * QW], F32, tag="rbr", name="rbr")
                        nc.vector.reciprocal_approx_fast(out=rbr[:], in_=rb[:])
                        for h in range(HL):
                            nc.vector.tensor_mul(
                                out=outt[h * D : (h + 1) * D, qs],
                                in0=outu[h][:, qs],
                                in1=rbr[:, h * QW : (h + 1) * QW],
                            )
                        for s2 in range(qh * (QW // 128), (qh + 1) * (QW // 128)):
                            for nck in range(C // 512):
                                bgl.append(
                                    (
                                        C_PROJ,
                                        lambda outt=outt, b0=b0, s2=s2, nck=nck: (
                                            emit_proj_chunk(outt, b0, s2, nck)
                                        ),
                                    )
                                )
                    return fin

                fin_cb = make_fin()
                for kc in range(KCN):
                    g = ci * KCN + kc
                    if kc + 1 < KCN:
                        nxt = emit_scores(b, qh, kc + 1, g + 1)
                    elif ci + 1 < len(chunks):
                        nb, nqh = chunks[ci + 1]
                        nxt = emit_scores(nb, nqh, 0, g + 1)
                    else:
                        nxt = None

                    # pt-pool WAR: the buffer exp(g) writes was read by the
                    # AVs of exp g-PTB+1; those must be emitted first
                    while pend_av and pend_av[0]["g"] <= g - (PTB - 1):
                        emit_av(pend_av.pop(0))

                    pt = ptp.tile([128, 2 * QW], MMDT, tag="pt")
                    nc.scalar.activation(
                        out=pt[:],
                        in_=pend[:],
                        func=mybir.ActivationFunctionType.Exp,
                        scale=SCALE,
                    )
                    e_end = (
                        max(
                            CLK["act"],
                            g_scores[g] + C_SEM,
                            av_done.get(g - PTB, 0.0) + C_SEM,
                        )
                        + C_EXP
                    )
                    CLK["act"] = e_end
                    exp_end[g] = e_end
                    pend_av.append(
                        {
                            "b": b, "kc": kc, "g": g, "pt": pt,
                            "vps": vps_cur, "fin": fin_cb if kc == KCN - 1 else None,
                        }
                    )

                    # first drain every ripe AV (its exp is comfortably done —
                    # the PE never stalls on these and they gate the exp
                    # chain via the pt-pool WAR horizon)
                    while pend_av and exp_end[pend_av[0]["g"]] <= CLK["pe"] - 150.0:
                        emit_av(pend_av.pop(0))

                    # credit-based background pumping: spread the remaining
                    # queue cost evenly over the remaining kcs, independent of
                    # the PE-vs-ACT clock drift (the scores pairs are
                    # priority-hoisted, so queueing background early cannot
                    # delay the exp chain; the PE FIFO just stays fed).
                    # Lazy proj chunks run only once the qkv queue is empty —
                    # they share the aux PSUM ring with the qkv accumulators.
                    rem_kc = max(len(chunks) * KCN - 1 - g, 1)
                    rem_cost = sum(it[1] for it in bg) + sum(c for c, _ in bgl)
                    credit = rem_cost / rem_kc
                    if ci == len(chunks) - 1:
                        credit = rem_cost
                    while credit > 0.0:
                        if pend_av and exp_end[pend_av[0]["g"]] <= CLK["pe"] - 150.0:
                            emit_av(pend_av.pop(0))
                        elif bg:
                            credit -= bg[0][1]
                            run_bg_item()
                        elif bgl:
                            credit -= bgl[0][0]
                            pump_lazy(1)
                        else:
                            break

                    pend = nxt

                if last:
                    # flush every remaining AV (and any leftover background)
                    flush_av()
                    while bg:
                        pump(1)
                    pump_lazy(len(bgl))
                    # tail: fine-grained per-s2 pipeline — PE ones-column
                    # matmul broadcasts the two denominators (fp32, ~0.4us
                    # cheaper than gpsimd partition_broadcast), recip, two
                    # muls, then this s2's projections and out-DMAs
                    nc.scalar.copy(out=rs2[:, qh, 0, :], in_=vps_cur[0][D : D + 1, :])
                    nc.vector.tensor_copy(
                        out=rs2[:, qh, 1, :], in_=vps_cur[1][D : D + 1, :]
                    )
                    for s2l in range(QW // 128):
                        s2 = qh * (QW // 128) + s2l
                        cs = slice(s2l * 128, (s2l + 1) * 128)
                        qsl = slice(
                            qh * QW + s2l * 128, qh * QW + (s2l + 1) * 128
                        )
                        rbp = ps_aux.tile([128, SC], F32, tag="aux", name="rbp")
                        nc.tensor.matmul(
                            rbp[:D, : HL * 128],
                            ones_row[:, :],
                            rs2[:, qh, :, s2l * 128 : (s2l + 1) * 128],
                            start=True,
                            stop=True,
                        )
                        rbr = rp.tile([D, HL * 128], F32, tag="rbrs", name="rbrs")
                        nc.vector.reciprocal_approx_fast(
                            out=rbr[:], in_=rbp[:D, : HL * 128]
                        )
                        for h in range(HL):
                            nc.vector.tensor_mul(
                                out=outt[h * D : (h + 1) * D, qsl],
                                in0=vps_cur[h][:D, cs],
                                in1=rbr[:, h * 128 : (h + 1) * 128],
                            )
                        # both nck projections into one (now free) score tile,
                        # then the two drain copies run on Scalar and Vector
                        # in parallel
                        pp2 = ps_st.tile([128, 2 * QW], F32, tag="st", name="tp")
                        for nck in range(C // 512):
                            nc.tensor.matmul(
                                pp2[:, nck * 512 : (nck + 1) * 512],
                                outt[:, s2 * 128 : (s2 + 1) * 128],
                                wp_sb[:, nck * 512 : (nck + 1) * 512],
                                start=True,
                                stop=True,
                            )
                        for nck in range(C // 512):
                            o_sb = op.tile([128, 512], MMDT, tag="o", name="o_sb")
                            src = pp2[:, nck * 512 : (nck + 1) * 512]
                            if nck == 0:
                                nc.scalar.copy(out=o_sb[:], in_=src)
                            else:
                                nc.vector.tensor_copy(out=o_sb[:], in_=src)
                            nc.sync.dma_start(
                                out_d.ap()[
                                    b0 + s2 * 128 : b0 + (s2 + 1) * 128,
                                    nck * 512 : (nck + 1) * 512,
                                ],
                                o_sb[:],
                            )

            # drain remaining background work
            flush_av()
            while bg:
                pump(1)
            pump_lazy(len(bgl))
    nc.compile()
    return nc


_NC_CACHE = {}


def _get_nc():
    if "nc" not in _NC_CACHE:
        _NC_CACHE["nc"] = build_nc()
    return _NC_CACHE["nc"]


def make_in_maps(x, w_qkv, w_proj):
    np_dt = mybir.dt.np(MMDT)
    x = np.asarray(x, dtype=np.float32)
    w_qkv = np.asarray(w_qkv, dtype=np.float32)
    w_proj = np.asarray(w_proj, dtype=np.float32)
    xt = np.ascontiguousarray(x.reshape(SEQ, C).T.astype(np_dt))
    in_maps = []
    for c in range(NCORES):
        cs = slice(128 * c, 128 * c + 128)
        wslice = np.concatenate(
            [w_qkv[:, cs], w_qkv[:, C:][:, cs], w_qkv[:, 2 * C :][:, cs]], axis=1
        ).astype(np_dt)
        # device expects [p, kt, m] row order (row = p*KT + kt)
        wslice = np.ascontiguousarray(
            wslice.reshape(KT, 128, MW).transpose(1, 0, 2).reshape(C, MW)
        )
        in_maps.append(
            {
                "xt": xt,
                "wqkv": wslice,
                "wproj": np.ascontiguousarray(w_proj[cs, :].astype(np_dt)),
            }
        )
    return in_maps


def kernel(x, w_qkv, w_proj, b_proj, _run_kwargs=None):
    # snapshot inputs to host numpy before any device/compile interaction
    in_maps = make_in_maps(x, w_qkv, w_proj)
    b_proj = np.asarray(b_proj, dtype=np.float32)
    nc = _get_nc()
    res = run_bass_kernel_spmd(
        nc, in_maps, core_ids=list(range(NCORES)), **(_run_kwargs or {})
    )
    acc = res.results[0]["out"].astype(np.float32)
    for c in range(1, NCORES):
        acc = acc + res.results[c]["out"]
    acc = acc + np.asarray(b_proj, dtype=np.float32)[None, :]
    out = acc.reshape(B, N, C)
    if _run_kwargs:
        kernel.last_result = res
    return out


# revision 27
# speedup vs baseline: 1.0531x; 1.0051x over previous
"""Multi-head attention block (B=2, N=2048, C=1024, H=16) on 8 TRN2 NeuronCores.

Sharding (tensor-parallel over heads): core c owns global heads {2c, 2c+1}:
  - w_qkv columns for q/k/v of those heads  -> [1024, 384] slice
  - w_proj rows for those heads             -> [128, 1024] slice
  - x replicated, pre-transposed on host to xT [1024, 4096] (and cast bf16)
Each core computes a full [4096, 1024] partial of the output projection;
the host sums the 8 partials and adds b_proj.

Device pipeline per core (bf16 matmuls, fp32 PSUM accumulation):
  1. qkvT = w_slice.T @ xT -> qT/kT/vT in [head_dim, seq] layout, emitted
     as single-kt quanta through a clock-budgeted background queue.
  2. Attention per (batch, 512-wide q chunk): both heads' scores^T
     [keys=128, 512] are packed into one [128, 1024] PSUM tile via
     row-group tile_position (the K=64 matmuls run concurrently in the
     PE array), one Exp per chunk on ScalarE (1/sqrt(d) folded into the
     activation scale; no max-subtraction needed for these O(1) scores),
     then a V-matmul per head whose [keys=128, 66] stationary operand is
     [v | ones] - the ones columns make the PSUM accumulator also
     collect softmax denominators.
  3. out^T chunks feed the projection matmul directly as lhsT (k=128,
     no transpose); results stream out per [128, 512] tile.
Scheduling: the emitter runs a static clock model of the PE and ACT
engines. The exp chain is the pacer; V-matmuls are DEFERRED (pt pool
bufs=8 gives ~8 kc of elastic lag) and emitted only when their exp is
predicted complete, with background work (qkv quanta, projection
chunks) pumped into the predicted PE slack so the PE never head-of-line
blocks on the exp. Deadline markers (pump_until) remain as the
correctness net for qkv/vaug availability.
V transposes: batch 0 builds vaug via PE transposes (the DMA ring is
busy streaming xT then); batch 1 uses two whole-batch DMA xbar
transposes on the by-then-quiet ring.
Tail: per-s2 pipeline where the denominator broadcast runs as a tiny
fp32 PE matmul (ones-column outer product) instead of the slow gpsimd
partition_broadcast, so the last out-DMAs leave ~1.5us after the final
V matmul.
"""

import math
import os

import numpy as np

os.environ.setdefault("JAX_PLATFORMS", "axon,cpu")

import concourse.mybir as mybir
import concourse.tile as tile
from concourse import bacc
from concourse.bass_utils import run_bass_kernel_spmd
from concourse.masks import make_identity

F32 = mybir.dt.float32
MMDT = mybir.dt.bfloat16  # matmul operand dtype

# Problem shape (hardcoded per contract)
B, N, C, H = 2, 2048, 1024, 16
D = C // H            # 64 head dim
SEQ = B * N           # 4096
NCORES = 8
HL = H // NCORES      # 2 local heads per core
MW = 3 * HL * D       # 384 w_qkv slice cols (q|k|v for 2 heads)
KT = C // 128         # 8 contraction tiles for the projections
SC = 512              # seq chunk for qkv stage
NSC = SEQ // SC       # 8
KCN = N // 128        # 16 key chunks per batch
QW = 512              # q-chunk width for attention
NQH = N // QW         # 4
SCALE = 1.0 / math.sqrt(D)
PTB = 8               # pt pool depth = max AV lag in kc

# static clock model costs (ns)
C_EXP = 1060          # ScalarE exp of [128, 1024] from PSUM
C_SEM = 150           # cross-engine semaphore latency
C_PAIR = 330          # row-tiled scores pair (LDW + MM 512)
C_AV = 235            # one AV matmul (LDW 66 + MM 512)
C_QKT = 230           # one qkv kt-matmul (FD 512)
C_PROJ = 240          # one proj chunk matmul (FD 512)
C_VAUG0 = 520         # b=0 vaug piece (4 PE transposes)
C_CHEAP = 40          # DVE-only / DMA-only quanta


def build_nc():
    nc = bacc.Bacc("TRN2", target_bir_lowering=False, debug=False)
    xt_d = nc.dram_tensor("xt", [C, SEQ], MMDT, kind="ExternalInput")
    wqkv_d = nc.dram_tensor("wqkv", [C, MW], MMDT, kind="ExternalInput")
    wproj_d = nc.dram_tensor("wproj", [HL * D, C], MMDT, kind="ExternalInput")
    out_d = nc.dram_tensor("out", [SEQ, C], MMDT, kind="ExternalOutput")

    with tile.TileContext(nc) as tc:
        with (
            tc.tile_pool(name="const", bufs=1) as const,
            tc.tile_pool(name="qkvt", bufs=1) as qkvt,
            tc.tile_pool(name="vaugp", bufs=2) as vaugp,
            tc.tile_pool(name="ptp", bufs=PTB) as ptp,
            tc.tile_pool(name="outt", bufs=2) as outtp,
            tc.tile_pool(name="rp", bufs=2) as rp,
            tc.tile_pool(name="op", bufs=5) as op,
            tc.tile_pool(name="ps_st", bufs=2, space="PSUM") as ps_st,
            tc.tile_pool(name="ps_v", bufs=1, space="PSUM") as ps_v,
            tc.tile_pool(name="ps_aux", bufs=2, space="PSUM") as ps_aux,
        ):
            # ---- constants ----
            junk = const.tile([128, 128], MMDT, tag="junk")
            ident = const.tile([128, 128], MMDT, tag="ident")
            ones_sb = const.tile([128, 1], F32, tag="ones")
            ones_row = const.tile([1, D], F32, tag="ones_row")
            w_sb = const.tile([128, KT, MW], MMDT, tag="wqkv")
            wp_sb = const.tile([128, C], MMDT, tag="wproj")
            tld = const.tile([1, 1], F32, tag="tld")
            nc.gpsimd.memset(junk[:], 1.0)
            nc.gpsimd.memset(ones_sb[:], 1.0)
            nc.gpsimd.memset(ones_row[:], 1.0)
            make_identity(nc, ident[:])
            # pre-trigger the exp ACT table load (~2.7us) during the initial
            # DMA wait so exp#0 doesn't pay it
            nc.scalar.activation(
                out=tld[:], in_=ones_sb[:1, :],
                func=mybir.ActivationFunctionType.Exp,
            )

            # persistent transposed qkv: [dim-of-2-heads=128, seq]
            q_sb = qkvt.tile([128, SEQ], MMDT, tag="q")
            k_sb = qkvt.tile([128, SEQ], MMDT, tag="k")
            v_sb = qkvt.tile([128, SEQ], MMDT, tag="v")
            dst = [q_sb, k_sb, v_sb]

            # full xT resident; chunk 0 kt-granular (so the first qkv
            # matmuls start as soon as their own kt slice lands) + weights
            # up front, the rest as whole chunks
            xt_all = qkvt.tile([128, KT, SEQ], MMDT, tag="xt_all")

            def xt_dma(sc):
                nc.sync.dma_start(
                    xt_all[:, :, sc * SC : (sc + 1) * SC],
                    xt_d.ap()[:, sc * SC : (sc + 1) * SC].rearrange(
                        "(kt p) n -> p kt n", p=128
                    ),
                )

            # input DMAs in need-order (ring processes doorbells in order):
            # k-weight columns first, then chunk 0 per-kt, q/v weights, proj
            # weights, and the rest
            def xt_dma_kts(sc):
                src = xt_d.ap()[:, sc * SC : (sc + 1) * SC].rearrange(
                    "(kt p) n -> p kt n", p=128
                )
                for kt in range(KT):
                    nc.sync.dma_start(
                        xt_all[:, kt, sc * SC : (sc + 1) * SC], src[:, kt, :]
                    )

            # w_qkv arrives host-pre-permuted to [p, kt, m] order so the whole
            # weight block is one 128 x 6KB-descriptor transfer (the old
            # (kt p) m layout generated 3072 tiny 256B descriptors)
            wsrc = wqkv_d.ap().rearrange("(p kt) m -> p kt m", p=128)
            nc.sync.dma_start(w_sb[:], wsrc)
            xt_dma_kts(0)
            xt_dma_kts(1)
            nc.sync.dma_start(wp_sb[:], wproj_d.ap())
            xt_dma_kts(2)
            xt_dma_kts(3)
            # b1 chunks paired: [128, kt, 1024] runs = 2KB descriptors
            for sc2 in (2, 3):
                nc.sync.dma_start(
                    xt_all[:, :, sc2 * 2 * SC : (sc2 + 1) * 2 * SC],
                    xt_d.ap()[:, sc2 * 2 * SC : (sc2 + 1) * 2 * SC].rearrange(
                        "(kt p) n -> p kt n", p=128
                    ),
                )

            # HAM warmup: keep the PE busy during the initial DMA wait so the
            # clock gate is at 8/8 when real work lands (junk operands — no
            # dependency on any DMA). Sized to end roughly when the first
            # xt/weight slices arrive.
            wu = ps_aux.tile([128, SC], F32, tag="aux", name="wu")
            for _ in range(52):
                nc.tensor.matmul(wu[:, :128], junk[:], junk[:], start=True, stop=True)

            # ---- static clock model ----
            CLK = {"pe": 0.0, "act": 0.0}
            exp_end = {}  # global exp index -> predicted completion ns

            # ---- background queue with markers ----
            bg = []   # deadline work: (key|None, cost, closure, opens_group)
            bgl = []  # lazy work (projection chunks): (cost, closure)
            done = set()
            # True while a multi-part qkv PSUM accumulation is mid-flight;
            # lazy proj chunks share the aux tag ring and must not land
            # between two parts of a live accumulation
            grp = {"open": False}

            def run_bg_item():
                key, cost, fn, opens = bg.pop(0)
                fn()
                CLK["pe"] += cost
                grp["open"] = opens
                if key is not None:
                    done.add(key)

            def pump(n=1):
                for _ in range(n):
                    if not bg:
                        return
                    run_bg_item()

            def pump_lazy(n=1):
                for _ in range(n):
                    if not bgl:
                        return
                    cost, fn = bgl.pop(0)
                    fn()
                    CLK["pe"] += cost

            def pump_until(key):
                while key not in done:
                    run_bg_item()

            # ---- qkv + vaug emission (all via bg) ----
            def emit_qkv_part(holder, sc, m, part, nparts=2):
                if part == 0:
                    holder["ps"] = ps_aux.tile([128, SC], F32, tag="aux", name="qkv_ps")
                ps = holder["ps"]
                step = KT // nparts
                for kt in range(part * step, (part + 1) * step):
                    nc.tensor.matmul(
                        ps[:],
                        w_sb[:, kt, m * 128 : (m + 1) * 128],
                        xt_all[:, kt, sc * SC : (sc + 1) * SC],
                        start=(kt == 0),
                        stop=(kt == KT - 1),
                    )
                if part == nparts - 1:
                    # all copies on DVE: the ScalarE queue must stay clear for
                    # the exp chain (sc==0 copies used to sit ahead of exp#0)
                    if sc == 0 and m == 1:
                        # first k block: land the kc=0 key chunk first so the
                        # very first scores matmul can start early
                        nc.vector.tensor_copy(out=dst[m][:, :128], in_=ps[:, :128])
                        nc.vector.tensor_copy(out=dst[m][:, 128:SC], in_=ps[:, 128:])
                    else:
                        nc.vector.tensor_copy(
                            out=dst[m][:, sc * SC : (sc + 1) * SC], in_=ps[:]
                        )

            vaug_store = {}
            vtb_store = {}
            MV = D + 2  # stationary width: v columns + ones (denominator) pad

            vtb_store[0] = vaugp.tile(
                [128, 16, D], MMDT, tag="vtb0", name="vtb", bufs=1
            )
            vtb_store[1] = vaugp.tile(
                [128, 16, D], MMDT, tag="vtb1", name="vtb", bufs=1
            )

            def emit_vtb_dma(h):
                # whole-batch transpose for b=1 on the (by then quiet) DMA
                # ring: vtb[p, T, d] = v[d, N + T*128 + p]
                nc.sync.dma_start_transpose(
                    vtb_store[h][:], v_sb[h * D : (h + 1) * D, N : 2 * N]
                )

            def emit_vaug_tr(holder, b, h, piece):
                b0 = b * N
                va = vaugp.tile([128, 4, MV], MMDT, tag=f"vaug{h}_{piece}", name="va")
                if b == 0 and piece < 3:
                    # JIT path: PE transpose (the DMA ring is busy streaming
                    # xt input during batch 0's first chunk)
                    tr = ps_aux.tile([128, 4 * D], MMDT, tag="aux", name="tr")
                    for tt in range(4):
                        t = piece * 4 + tt
                        nc.tensor.transpose(
                            tr[:, tt * D : (tt + 1) * D],
                            v_sb[h * D : (h + 1) * D, b0 + t * 128 : b0 + (t + 1) * 128],
                            ident[h * D : (h + 1) * D, h * D : (h + 1) * D],
                        )
                    nc.vector.tensor_copy(
                        out=va[:, :, :D], in_=tr[:].rearrange("p (a d) -> p a d", a=4)
                    )
                elif b == 0:
                    # piece 3 is needed late enough (kc12) that its transpose
                    # can ride the ring right after the xt stream drains;
                    # stage through vtb (b=1's later full-tile DMA overwrites)
                    nc.sync.dma_start_transpose(
                        vtb_store[h][:, 12:16, :],
                        v_sb[h * D : (h + 1) * D, b0 + 1536 : b0 + 2048],
                    )
                    nc.vector.tensor_copy(
                        out=va[:, :, :D], in_=vtb_store[h][:, 12:16, :]
                    )
                else:
                    nc.vector.tensor_copy(
                        out=va[:, :, :D],
                        in_=vtb_store[h][:, piece * 4 : (piece + 1) * 4, :],
                    )
                nc.vector.tensor_copy(
                    out=va[:, :, D : D + 2],
                    in_=ones_sb[:, None, :].to_broadcast([128, 4, 2]),
                )
                vaug_store[(b, h, piece)] = va

            def add_qkv(b, m, scl, nparts=2):
                sc = b * (NSC // 2) + scl
                holder = {}
                for part in range(nparts):
                    key = None
                    if part == nparts - 1:
                        key = ("kvq"[0 if m == 1 else 1 if m == 2 else 2], b, scl)
                    bg.append(
                        (
                            key,
                            C_QKT * (KT // nparts),
                            lambda sc=sc, m=m, part=part, holder=holder, np_=nparts: (
                                emit_qkv_part(holder, sc, m, part, np_)
                            ),
                            part < nparts - 1,
                        )
                    )

            def add_vaug(b, scl):
                for h in range(HL):
                    bg.append(
                        (
                            ("vaug", b, h, scl),
                            C_VAUG0 if (b == 0 and scl < 3) else C_CHEAP,
                            lambda b=b, h=h, scl=scl: emit_vaug_tr({}, b, h, scl),
                            False,
                        )
                    )

            # b=0: block 0 ordered for the fastest first exp (k, q, then v);
            # k-blocks early (scores need them at kc=4*blk), q for the next
            # chunk pulled ahead of late v-blocks so chunk transitions don't
            # burst
            add_qkv(0, 1, 0)
            add_qkv(0, 0, 0)
            add_qkv(0, 1, 1)
            add_qkv(0, 2, 0)
            add_vaug(0, 0)
            add_qkv(0, 1, 2)
            add_qkv(0, 2, 1)
            add_vaug(0, 1)
            add_qkv(0, 1, 3)
            add_qkv(0, 2, 2)
            add_vaug(0, 2)
            add_qkv(0, 0, 1)
            add_qkv(0, 2, 3)
            add_vaug(0, 3)
            add_qkv(0, 0, 2)
            add_qkv(0, 0, 3)
            # b=1: k/v first, whole-batch vaug transposes ride the (by now
            # quiet) DMA ring, q spread between
            add_qkv(1, 1, 0)
            add_qkv(1, 0, 0)
            add_qkv(1, 2, 0)
            add_qkv(1, 1, 1)
            add_qkv(1, 2, 1)
            add_qkv(1, 1, 2)
            add_qkv(1, 2, 2)
            add_qkv(1, 0, 1)
            add_qkv(1, 1, 3)
            add_qkv(1, 2, 3)
            # vtb DMA transposes read ALL of b=1's v — they must come after
            # every b=1 v-block is queued (the tracker cannot wait on writes
            # emitted later in program order)
            bg.append((None, C_CHEAP, lambda: emit_vtb_dma(0), False))
            bg.append((None, C_CHEAP, lambda: emit_vtb_dma(1), False))
            add_vaug(1, 0)
            add_vaug(1, 1)
            add_qkv(1, 0, 2)
            add_vaug(1, 2)
            add_vaug(1, 3)
            add_qkv(1, 0, 3)

            def emit_proj_chunk(outt, b0, s2, nck):
                pp = ps_aux.tile([128, 512], F32, tag="aux", name="proj_ps")
                nc.tensor.matmul(
                    pp[:],
                    outt[:, s2 * 128 : (s2 + 1) * 128],
                    wp_sb[:, nck * 512 : (nck + 1) * 512],
                    start=True,
                    stop=True,
                )
                o_sb = op.tile([128, 512], MMDT, tag="o", name="o_sb")
                nc.vector.tensor_copy(out=o_sb[:], in_=pp[:])
                nc.sync.dma_start(
                    out_d.ap()[
                        b0 + s2 * 128 : b0 + (s2 + 1) * 128,
                        nck * 512 : (nck + 1) * 512,
                    ],
                    o_sb[:],
                )

            # ---- attention ----
            g_scores = {}  # global kc index -> predicted scores-done ns

            def emit_scores(b, qh, kc, g):
                b0 = b * N
                q0 = b0 + qh * QW
                pump_until(("q", b, qh))
                pump_until(("k", b, kc // 4))
                st = ps_st.tile([128, 2 * QW], F32, tag="st", name="st")
                # top priority: the exp chain paces the kernel; splitting the
                # row-packed pair stalls the exp at chunk boundaries
                with tc.high_priority():
                    for h in range(HL):
                        hs = slice(h * D, (h + 1) * D)
                        nc.tensor.matmul(
                            st[:, h * QW : (h + 1) * QW],
                            k_sb[hs, b0 + kc * 128 : b0 + (kc + 1) * 128],
                            q_sb[hs, q0 : q0 + QW],
                            start=True,
                            stop=True,
                            tile_position=(h * D, 0),
                        )
                # st WAR: this buffer was read by exp g-2
                CLK["pe"] = max(CLK["pe"], exp_end.get(g - 2, 0.0)) + C_PAIR
                g_scores[g] = CLK["pe"]
                return st

            # deferred AV queue: entries emitted only when their exp is
            # predicted done (or forced by pt-pool WAR / chunk drain)
            pend_av = []
            av_done = {}  # global exp index -> PE completion of its AVs

            def emit_av(e):
                b, kc = e["b"], e["kc"]
                for h in range(HL):
                    pump_until(("vaug", b, h, kc // 4))
                for h in range(HL):
                    nc.tensor.matmul(
                        e["vps"][h][:MV, :],
                        vaug_store[(b, h, kc // 4)][:, kc % 4, :],
                        e["pt"][:, h * QW : (h + 1) * QW],
                        start=(kc == 0),
                        stop=(kc == KCN - 1),
                    )
                CLK["pe"] = max(CLK["pe"], exp_end[e["g"]]) + 2 * C_AV
                av_done[e["g"]] = CLK["pe"]
                if e["fin"] is not None:
                    e["fin"]()

            def flush_av(n=None):
                cnt = len(pend_av) if n is None else n
                for _ in range(cnt):
                    if not pend_av:
                        return
                    emit_av(pend_av.pop(0))

            chunks = [(b, qh) for b in range(B) for qh in range(NQH)]
            pend = emit_scores(0, 0, 0, 0)
            outt = outu = None
            rs2 = None
            vps_cur = None
            for ci, (b, qh) in enumerate(chunks):
                b0 = b * N
                last = ci == len(chunks) - 1
                if qh == 0:
                    outt = outtp.tile([128, N], MMDT, tag="outT", name="outt")
                    outu = [
                        outtp.tile([D, N], MMDT, tag=f"outu{h}", name="outu")
                        for h in range(HL)
                    ]
                    # [1, qh, h, q] so each chunk's denominator block is
                    # contiguous (2D-viewable for partition_broadcast)
                    rs2 = rp.tile([1, NQH, HL, QW], F32, tag="rs2", name="rs2")
                vps_cur = [
                    ps_v.tile([128, QW], F32, tag=f"vps{h}", name=f"vps{h}")
                    for h in range(HL)
                ]
                qs = slice(qh * QW, (qh + 1) * QW)

                # per-chunk finalizer, attached to the last AV of the chunk:
                # drain vps to SBUF, normalize off the critical path, queue
                # this q-range's projection chunks as lazy work
                def make_fin(
                    b=b, qh=qh, b0=b0, outt=outt, outu=outu, rs2=rs2,
                    vps=vps_cur, qs=qs, last=last,
                ):
                    def fin():
                        if last:
                            return  # tail handled inline below
                        for h in range(HL):
                            nc.vector.tensor_copy(out=outu[h][:, qs], in_=vps[h][:D, :])
                            nc.vector.tensor_copy(
                                out=rs2[:, qh, h, :], in_=vps[h][D : D + 1, :]
                            )
                        # one broadcast+recip covers both heads' denominators
                        rb = rp.tile([D, HL * QW], F32, tag="rb", name="rb")
                        nc.gpsimd.partition_broadcast(
                            rb[:], rs2[:, qh].rearrange("p h q -> p (h q)")
                        )
                        rbr = rp.tile([D, HL * QW], F32, tag="rbr", name="rbr")
                        nc.vector.reciprocal_approx_fast(out=rbr[:], in_=rb[:])
                        for h in range(HL):
                            nc.vector.tensor_mul(
                                out=outt[h * D : (h + 1) * D, qs],
                                in0=outu[h][:, qs],
                                in1=rbr[:, h * QW : (h + 1) * QW],
                            )
                        for s2 in range(qh * (QW // 128), (qh + 1) * (QW // 128)):
                            for nck in range(C // 512):
                                bgl.append(
                                    (
                                        C_PROJ,
                                        lambda outt=outt, b0=b0, s2=s2, nck=nck: (
                                            emit_proj_chunk(outt, b0, s2, nck)
                                        ),
                                    )
                                )
                    return fin

                fin_cb = make_fin()
                for kc in range(KCN):
                    g = ci * KCN + kc
                    if kc + 1 < KCN:
                        nxt = emit_scores(b, qh, kc + 1, g + 1)
                    elif ci + 1 < len(chunks):
                        nb, nqh = chunks[ci + 1]
                        nxt = emit_scores(nb, nqh, 0, g + 1)
                    else:
                        nxt = None

                    # pt-pool WAR: the buffer exp(g) writes was read by the
                    # AVs of exp g-PTB+1; those must be emitted first
                    while pend_av and pend_av[0]["g"] <= g - (PTB - 1):
                        emit_av(pend_av.pop(0))

                    pt = ptp.tile([128, 2 * QW], MMDT, tag="pt")
                    nc.scalar.activation(
                        out=pt[:],
                        in_=pend[:],
                        func=mybir.ActivationFunctionType.Exp,
                        scale=SCALE,
                    )
                    e_end = (
                        max(
                            CLK["act"],
                            g_scores[g] + C_SEM,
                            av_done.get(g - PTB, 0.0) + C_SEM,
                        )
                        + C_EXP
                    )
                    CLK["act"] = e_end
                    exp_end[g] = e_end
                    pend_av.append(
                        {
                            "b": b, "kc": kc, "g": g, "pt": pt,
                            "vps": vps_cur, "fin": fin_cb if kc == KCN - 1 else None,
                        }
                    )

                    # first drain every ripe AV (its exp is comfortably done —
                    # the PE never stalls on these and they gate the exp
                    # chain via the pt-pool WAR horizon)
                    while pend_av and exp_end[pend_av[0]["g"]] <= CLK["pe"] - 150.0:
                        emit_av(pend_av.pop(0))

                    # credit-based background pumping: spread the remaining
                    # queue cost evenly over the remaining kcs, independent of
                    # the PE-vs-ACT clock drift (the scores pairs are
                    # priority-hoisted, so queueing background early cannot
                    # delay the exp chain; the PE FIFO just stays fed).
                    # Lazy proj chunks run only once the qkv queue is empty —
                    # they share the aux PSUM ring with the qkv accumulators.
                    rem_kc = max(len(chunks) * KCN - 1 - g, 1)
                    rem_cost = sum(it[1] for it in bg) + sum(c for c, _ in bgl)
                    credit = rem_cost / rem_kc
                    while credit > 0.0:
                        if pend_av and exp_end[pend_av[0]["g"]] <= CLK["pe"] - 150.0:
                            emit_av(pend_av.pop(0))
                        elif bg:
                            credit -= bg[0][1]
                            run_bg_item()
                        elif bgl:
                            credit -= bgl[0][0]
                            pump_lazy(1)
                        else:
                            break

                    pend = nxt

                if last:
                    # flush every remaining AV (and any leftover background)
                    flush_av()
                    while bg:
                        pump(1)
                    pump_lazy(len(bgl))
                    # tail: fine-grained per-s2 pipeline — PE ones-column
                    # matmul broadcasts the two denominators (fp32, ~0.4us
                    # cheaper than gpsimd partition_broadcast), recip, two
                    # muls, then this s2's projections and out-DMAs
                    nc.scalar.copy(out=rs2[:, qh, 0, :], in_=vps_cur[0][D : D + 1, :])
                    nc.vector.tensor_copy(
                        out=rs2[:, qh, 1, :], in_=vps_cur[1][D : D + 1, :]
                    )
                    for s2l in range(QW // 128):
                        s2 = qh * (QW // 128) + s2l
                        cs = slice(s2l * 128, (s2l + 1) * 128)
                        qsl = slice(
                            qh * QW + s2l * 128, qh * QW + (s2l + 1) * 128
                        )
                        rbp = ps_aux.tile([128, SC], F32, tag="aux", name="rbp")
                        nc.tensor.matmul(
                            rbp[:D, : HL * 128],
                            ones_row[:, :],
                            rs2[:, qh, :, s2l * 128 : (s2l + 1) * 128],
                            start=True,
                            stop=True,
                        )
                        rbr = rp.tile([D, HL * 128], F32, tag="rbrs", name="rbrs")
                        nc.vector.reciprocal_approx_fast(
                            out=rbr[:], in_=rbp[:D, : HL * 128]
                        )
                        for h in range(HL):
                            nc.vector.tensor_mul(
                                out=outt[h * D : (h + 1) * D, qsl],
                                in0=vps_cur[h][:D, cs],
                                in1=rbr[:, h * 128 : (h + 1) * 128],
                            )
                        # both nck projections into one (now free) score tile,
                        # then the two drain copies run on Scalar and Vector
                        # in parallel
                        pp2 = ps_st.tile([128, 2 * QW], F32, tag="st", name="tp")
                        for nck in range(C // 512):
                            nc.tensor.matmul(
                                pp2[:, nck * 512 : (nck + 1) * 512],
                                outt[:, s2 * 128 : (s2 + 1) * 128],
                                wp_sb[:, nck * 512 : (nck + 1) * 512],
                                start=True,
                                stop=True,
                            )
                        for nck in range(C // 512):
                            o_sb = op.tile([128, 512], MMDT, tag="o", name="o_sb")
                            src = pp2[:, nck * 512 : (nck + 1) * 512]
                            if nck == 0:
                                nc.scalar.copy(out=o_sb[:], in_=src)
                            else:
                                nc.vector.tensor_copy(out=o_sb[:], in_=src)
                            nc.sync.dma_start(
                                out_d.ap()[
                                    b0 + s2 * 128 : b0 + (s2 + 1) * 128,
                                    nck * 512 : (nck + 1) * 512,
                                ],
                                o_sb[:],
                            )

            # drain remaining background work
            flush_av()
            while bg:
                pump(1)
            pump_lazy(len(bgl))
    nc.compile()
    return nc


_NC_CACHE = {}


def _get_nc():
    if "nc" not in _NC_CACHE:
        _NC_CACHE["nc"] = build_nc()
    return _NC_CACHE["nc"]


def make_in_maps(x, w_qkv, w_proj):
    np_dt = mybir.dt.np(MMDT)
    x = np.asarray(x, dtype=np.float32)
    w_qkv = np.asarray(w_qkv, dtype=np.float32)
    w_proj = np.asarray(w_proj, dtype=np.float32)
    xt = np.ascontiguousarray(x.reshape(SEQ, C).T.astype(np_dt))
    in_maps = []
    for c in range(NCORES):
        cs = slice(128 * c, 128 * c + 128)
        wslice = np.concatenate(
            [w_qkv[:, cs], w_qkv[:, C:][:, cs], w_qkv[:, 2 * C :][:, cs]], axis=1
        ).astype(np_dt)
        # device expects [p, kt, m] row order (row = p*KT + kt)
        wslice = np.ascontiguousarray(
            wslice.reshape(KT, 128, MW).transpose(1, 0, 2).reshape(C, MW)
        )
        in_maps.append(
            {
                "xt": xt,
                "wqkv": wslice,
                "wproj": np.ascontiguousarray(w_proj[cs, :].astype(np_dt)),
            }
        )
    return in_maps


def kernel(x, w_qkv, w_proj, b_proj, _run_kwargs=None):
    # snapshot inputs to host numpy before any device/compile interaction
    in_maps = make_in_maps(x, w_qkv, w_proj)
    b_proj = np.asarray(b_proj, dtype=np.float32)
    nc = _get_nc()
    res = run_bass_kernel_spmd(
        nc, in_maps, core_ids=list(range(NCORES)), **(_run_kwargs or {})
    )
    acc = res.results[0]["out"].astype(np.float32)
    for c in range(1, NCORES):
        acc = acc + res.results[c]["out"]
    acc = acc + np.asarray(b_proj, dtype=np.float32)[None, :]
    out = acc.reshape(B, N, C)
    if _run_kwargs:
        kernel.last_result = res
    return out
